# revision 1
# baseline (speedup 1.0000x reference)
"""Trainium2 Bass kernel for causal self-attention (GQA, RoPE, q/k-RMSNorm).

Sharding: tensor-parallel over heads across 8 cores.
  - core c owns q-heads [4c, 4c+4) and kv-head c//2 (each kv head serves 8 q heads)
  - x^T is built locally on each core via DMA-transpose (bf16) and kept in SBUF
  - attention is computed transposed (E^T = exp(K·Q^T)) so V in natural [S,D]
    layout is the matmul lhsT and y^T comes out in [D,T] layout directly
  - y^T is AllGathered per head (4 collectives overlapped with attention);
    o_proj is column-sharded: core c computes Wo[256c:256c+256,:] @ y^T_full
  - head-dim rows of q/k are interleaved (d -> [0,64,1,65,...]) so the RoPE
    rotate-half becomes an adjacent-pair partition swap (one stream_shuffle)
  - rmsnorm scale and the norm weight are applied in one shot: the PE
    broadcast matmul computes w[p] * rinv[t] (lhsT = w row, rhs = 1/rms row)

Matmul dtypes: QKV + o_proj in bf16 (fp32 PSUM accum), attention in float32r.
"""

import sys

sys.path.insert(0, "/opt/trn_rl_repo")

from contextlib import ExitStack

import numpy as np

import bass_rust
import concourse.bass as bass
import concourse.mybir as mybir
from concourse import tile

F32 = mybir.dt.float32
F32R = mybir.dt.float32r
BF16 = mybir.dt.bfloat16

N_HEAD = 32
N_KV = 4
D = 128
C = 2048
T = 2048
NCORES = 8
HPC = N_HEAD // NCORES  # q heads per core = 4
THETA = 1000000.0
EPS = 1e-6
SCALE = 1.0 / np.sqrt(128.0)

NT = T // 512  # 4 T-chunks of 512
NK = C // 128  # 16 contraction tiles for qkv
NS = T // 128  # 16 S-blocks of 128

# stream_shuffle swaps within each 32-partition quadrant; adjacent-pair swap
SWAP_MASK = [i ^ 1 for i in range(32)]

_BF16_NP = None


def _bf16():
    global _BF16_NP
    if _BF16_NP is None:
        import ml_dtypes

        _BF16_NP = np.dtype(ml_dtypes.bfloat16)
    return _BF16_NP


def split_multiwaits(nc):
    """The walrus build in this container supports one sync-wait per
    instruction; hoist extra waits onto NOPs inserted before the offender."""
    ctr = 0
    for f in nc.m.functions:
        for bb in f.blocks:
            new_insts = []
            changed = False
            for inst in bb.instructions:
                si = inst.sync_info
                if si is not None and si.on_wait and len(si.on_wait) > 1:
                    waits = list(si.on_wait)
                    for w in waits[:-1]:
                        ctr += 1
                        nop = bass_rust.InstNoOp(name=f"splitw-{ctr}", ins=[], outs=[])
                        nop.engine = inst.engine
                        nop.sync_info = bass_rust.SyncInfo(on_wait=[w], on_update=[])
                        new_insts.append(nop)
                    inst.sync_info = bass_rust.SyncInfo(
                        on_wait=[waits[-1]], on_update=list(si.on_update or [])
                    )
                    changed = True
                new_insts.append(inst)
            if changed:
                bb.instructions = new_insts


def build_program(bench_reps=0, phases="ABDF"):
    nc = bass.Bass("TRN2", target_bir_lowering=False, debug=False, num_devices=NCORES)

    xb = nc.declare_dram_parameter("xb", [T, C], BF16, isOutput=False)
    wq = nc.declare_dram_parameter("wq", [128, HPC * NK * 128], BF16, isOutput=False)
    wk = nc.declare_dram_parameter("wk", [128, NK * 128], BF16, isOutput=False)
    wv = nc.declare_dram_parameter("wv", [128, NK * 128], BF16, isOutput=False)
    wo = nc.declare_dram_parameter("wo", [128, 32 * 256], BF16, isOutput=False)
    cost = nc.declare_dram_parameter("cost", [128, T], F32, isOutput=False)
    sint = nc.declare_dram_parameter("sint", [128, T], F32, isOutput=False)
    wqn = nc.declare_dram_parameter("wqn", [1, 128], F32, isOutput=False)
    wkn = nc.declare_dram_parameter("wkn", [1, 128], F32, isOutput=False)
    identp = nc.declare_dram_parameter("identp", [128, 128], BF16, isOutput=False)
    maskp = nc.declare_dram_parameter("maskp", [128, 896], BF16, isOutput=False)
    outT = nc.declare_dram_parameter("outT", [256, T], F32, isOutput=True)

    rg = [list(range(NCORES))]
    collectives = bench_reps == 0

    with tile.TileContext(nc) as tc, ExitStack() as ctx:
        const = ctx.enter_context(tc.tile_pool(name="const", bufs=1))
        wpool = ctx.enter_context(tc.tile_pool(name="wpool", bufs=1))
        act = ctx.enter_context(tc.tile_pool(name="act", bufs=1))
        dram = ctx.enter_context(tc.tile_pool(name="dram", bufs=1, space="DRAM"))

        # ---- constants ----
        ones128 = const.tile([128, 128], F32)
        nc.vector.memset(ones128[:], 1.0)
        ones_col = const.tile([128, 1], F32R)
        nc.vector.tensor_copy(ones_col[:], ones128[:, 0:1])
        ones_row = const.tile([1, 128], F32R)
        nc.vector.tensor_copy(ones_row[:], ones128[0:1, :])
        eps_col = const.tile([128, 1], F32)
        nc.vector.memset(eps_col[:], EPS)
        ones_colb = const.tile([128, 1], BF16)
        nc.vector.memset(ones_colb[:], 1.0)
        identb = const.tile([128, 128], BF16)
        nc.sync.dma_start(identb[:], identp[:, :])
        # one wide causal-mask tile; diagonal-block mask u is the slice
        # mask_big[:, (3-u)*128 : (3-u)*128+512]  (keep iff f - p - 128u >= 0)
        mask_big = const.tile([128, 896], BF16)
        nc.sync.dma_start(mask_big[:], maskp[:, :])
        masks = [mask_big[:, (3 - u) * 128:(3 - u) * 128 + 512] for u in range(4)]

        # ---- resident weights / tables ----
        skip_w = "W" in phases
        wq_sb = wpool.tile([128, HPC * NK * 128], BF16)
        (None if skip_w else nc.sync.dma_start(wq_sb[:], wq[:, :]))
        wk_sb = wpool.tile([128, NK * 128], BF16)
        (None if skip_w else nc.sync.dma_start(wk_sb[:], wk[:, :]))
        wv_sb = wpool.tile([128, NK * 128], BF16)
        (None if skip_w else nc.sync.dma_start(wv_sb[:], wv[:, :]))
        cos_sb = wpool.tile([128, T], F32)
        (None if skip_w else nc.sync.dma_start(cos_sb[:], cost[:, :]))
        sin_sb = wpool.tile([128, T], F32)
        (None if skip_w else nc.sync.dma_start(sin_sb[:], sint[:, :]))
        wqn_f = wpool.tile([1, 128], F32)
        (None if skip_w else nc.sync.dma_start(wqn_f[:], wqn[:, :]))
        wkn_f = wpool.tile([1, 128], F32)
        (None if skip_w else nc.sync.dma_start(wkn_f[:], wkn[:, :]))
        wqn_sb = wpool.tile([1, 128], F32R)
        nc.vector.tensor_copy(wqn_sb[:], wqn_f[:])
        wkn_sb = wpool.tile([1, 128], F32R)
        nc.vector.tensor_copy(wkn_sb[:], wkn_f[:])

        # ---- persistent activations ----
        qT = [act.tile([128, T], F32R, name=f"qT{h}") for h in range(HPC)]
        kT = act.tile([128, T], F32R)
        vN = act.tile([128, NS * 128], BF16)  # natural [S,D] as 16 s-tiles
        yT = [act.tile([128, T], BF16, name=f"yT{h}") for h in range(HPC)]

        # DRAM bounce + collective buffers
        y_in = [dram.tile([128, T], BF16, name=f"yin{h}") for h in range(HPC)]
        yt_all = [
            dram.tile(
                [NCORES * 128, T], BF16, name=f"ytall{h}",
                addr_space="Shared" if collectives else "Local",
            )
            for h in range(HPC)
        ]

        def body():
            # ===== Phase A: x^T via DMA transpose (bf16), kept in SBUF =====
            with tc.tile_pool(name="xtp", bufs=1) as xtp:
                xT = [xtp.tile([128, T], BF16, name=f"xT{k}") for k in range(NK)]
                if "A" in phases:
                    natiles = 4 if "A4" in phases else 16
                    with tc.tile_pool(name="pa_sb", bufs=2) as pa_sb, \
                         tc.tile_pool(name="pa_ps", bufs=4, space="PSUM") as pa_ps:
                        for tt in range(natiles):
                            xtile = pa_sb.tile([128, T], BF16, tag="xtile")
                            nc.sync.dma_start(
                                xtile[:], xb[tt * 128:(tt + 1) * 128, :]
                            )
                            for k in range(NK):
                                pt = pa_ps.tile([128, 128], BF16, tag="pt")
                                nc.tensor.transpose(
                                    pt[:], xtile[:, k * 128:(k + 1) * 128], identb[:]
                                )
                                nc.vector.tensor_copy(
                                    xT[k][:, tt * 128:(tt + 1) * 128], pt[:]
                                )
                if "B" not in phases:
                    return

                # ===== Phase B+C: QKV + RMSNorm + RoPE =====
                with tc.tile_pool(name="pc_sb", bufs=2) as pc_sb, \
                     tc.tile_pool(name="pb_ps", bufs=1, space="PSUM") as pb_ps, \
                     tc.tile_pool(name="pc_ps", bufs=2, space="PSUM") as pc_ps:

                    def norm_rope(ps, w_row, j, dest):
                        js = slice(j * 512, (j + 1) * 512)
                        raw = pc_sb.tile([128, 512], F32, tag="cA")
                        nc.vector.tensor_copy(raw[:], ps[:])
                        sqr = pc_sb.tile([128, 512], F32R, tag="cB")
                        nc.vector.tensor_mul(sqr[:], raw[:], raw[:])
                        ssq = pc_ps.tile([128, 512], F32, tag="cps")
                        nc.tensor.matmul(ssq[0:1, :], ones_col[:], sqr[:])
                        rms = pc_sb.tile([1, 512], F32, tag="cC")
                        nc.scalar.activation(
                            rms[:], ssq[0:1, :], mybir.ActivationFunctionType.Sqrt,
                            scale=1.0 / 128.0, bias=eps_col[0:1, :],
                        )
                        rinv = pc_sb.tile([1, 512], F32R, tag="cC")
                        with nc.allow_low_precision(reason="feeds PE broadcast"):
                            nc.vector.reciprocal(rinv[:], rms[:])
                        # rb[p,t] = w[p] * rinv[t]  (rank-1 PE broadcast)
                        rb = pc_ps.tile([128, 512], F32, tag="cps")
                        nc.tensor.matmul(rb[:], w_row[:], rinv[:])
                        qn = pc_sb.tile([128, 512], F32, tag="cB")
                        nc.vector.tensor_mul(qn[:], raw[:], rb[:])
                        qs = pc_sb.tile([128, 512], F32, tag="cA")
                        nc.vector.stream_shuffle(qs[:], qn[:], mask=SWAP_MASK)
                        t1 = pc_sb.tile([128, 512], F32, tag="cC")
                        nc.vector.tensor_mul(t1[:], qn[:], cos_sb[:, js])
                        t2 = pc_sb.tile([128, 512], F32, tag="cB")
                        nc.vector.tensor_mul(t2[:], qs[:], sin_sb[:, js])
                        nc.vector.tensor_add(dest[:, js], t1[:], t2[:])

                    for j in range(NT):
                        js = slice(j * 512, (j + 1) * 512)
                        ps_q = [
                            pb_ps.tile([128, 512], F32, tag=f"psq{h}", name=f"psq{h}")
                            for h in range(HPC)
                        ]
                        ps_k = pb_ps.tile([128, 512], F32, tag="psk")
                        ps_v = pb_ps.tile([128, 512], F32, tag="psv")
                        for k in range(NK):
                            st = dict(start=(k == 0), stop=(k == NK - 1))
                            rhs = xT[k][:, js]
                            for h in range(HPC):
                                nc.tensor.matmul(
                                    ps_q[h][:],
                                    wq_sb[:, (h * NK + k) * 128:(h * NK + k + 1) * 128],
                                    rhs, **st,
                                )
                            nc.tensor.matmul(
                                ps_k[:], wk_sb[:, k * 128:(k + 1) * 128], rhs, **st
                            )
                            nc.tensor.matmul(
                                ps_v[:], wv_sb[:, k * 128:(k + 1) * 128], rhs, **st
                            )
                        for h in range(HPC):
                            norm_rope(ps_q[h], wqn_sb, j, qT[h])
                        norm_rope(ps_k, wkn_sb, j, kT)
                        # v: transpose [D,T]-chunk into natural [S,D] tiles
                        vt = pc_sb.tile([128, 512], BF16, tag="cA")
                        nc.vector.tensor_copy(vt[:], ps_v[:])
                        for u in range(4):
                            s_tile = j * 4 + u
                            pvt = pc_ps.tile([128, 512], BF16, tag="cps")
                            nc.tensor.transpose(
                                pvt[:, 0:128], vt[:, u * 128:(u + 1) * 128], identb[:]
                            )
                            nc.vector.tensor_copy(
                                vN[:, s_tile * 128:(s_tile + 1) * 128], pvt[:, 0:128]
                            )

            # ===== Phase D: attention (+ per-head y AllGather) =====
            if "D" not in phases:
                return
            with tc.tile_pool(name="pd_sb", bufs=3) as pd_sb, \
                 tc.tile_pool(name="pd_ps", bufs=1, space="PSUM") as pd_ps, \
                 tc.tile_pool(name="ps_ps", bufs=2, space="PSUM") as ps_ps:
                for h in range(HPC):
                    for j in range(NT):
                        js = slice(j * 512, (j + 1) * 512)
                        nblk = 4 * j + 4
                        ps_y = pd_ps.tile([128, 512], F32, tag="psy")
                        ps_den = pd_ps.tile([128, 512], F32, tag="psden")
                        for i in range(nblk):
                            ps_s = ps_ps.tile([128, 512], F32, tag="pss")
                            nc.tensor.matmul(
                                ps_s[:], kT[:, i * 128:(i + 1) * 128], qT[h][:, js]
                            )
                            et = pd_sb.tile([128, 512], BF16, tag="et")
                            nc.scalar.activation(
                                et[:], ps_s[:], mybir.ActivationFunctionType.Exp,
                                scale=float(SCALE),
                            )
                            if i >= 4 * j:  # diagonal block: causal mask
                                etm = pd_sb.tile([128, 512], BF16, tag="etm")
                                nc.vector.tensor_mul(
                                    etm[:], et[:], masks[i - 4 * j]
                                )
                                et = etm
                            st = dict(start=(i == 0), stop=(i == nblk - 1))
                            nc.tensor.matmul(
                                ps_y[:], vN[:, i * 128:(i + 1) * 128], et[:], **st
                            )
                            nc.tensor.matmul(
                                ps_den[0:1, :], ones_colb[:], et[:], **st
                            )
                        rd = pd_sb.tile([1, 512], F32R, tag="rd")
                        with nc.allow_low_precision(reason="feeds PE broadcast"):
                            nc.vector.reciprocal(rd[:], ps_den[0:1, :])
                        ps_rb = pd_ps.tile([128, 512], F32, tag="psrb")
                        nc.tensor.matmul(ps_rb[:], ones_row[:], rd[:])
                        ytmp = pd_sb.tile([128, 512], F32, tag="ytmp")
                        nc.vector.tensor_copy(ytmp[:], ps_y[:])
                        nc.vector.tensor_mul(yT[h][:, js], ytmp[:], ps_rb[:])
                    # gather this head's y^T across cores
                    nc.sync.dma_start(y_in[h][:, :], yT[h][:])
                    if collectives:
                        nc.gpsimd.collective_compute(
                            "AllGather", mybir.AluOpType.bypass, replica_groups=rg,
                            ins=[y_in[h][:].opt()], outs=[yt_all[h][:].opt()],
                        )

            # ===== Phase F: o_proj (column shard) =====
            if "F" not in phases:
                return
            with tc.tile_pool(name="pf_sb", bufs=3) as pf_sb, \
                 tc.tile_pool(name="pf_ps", bufs=1, space="PSUM") as pf_ps:
                ps_o = [
                    [
                        pf_ps.tile([128, 512], F32, tag=f"pso{m}{j}", name=f"pso{m}{j}")
                        for j in range(NT)
                    ]
                    for m in range(2)
                ]
                for h in range(HPC):
                    for cp in range(NCORES):
                        k = 4 * cp + h  # global head index = wo k-tile index
                        yk = pf_sb.tile([128, T], BF16, tag="yk")
                        nc.sync.dma_start(
                            yk[:], yt_all[h][cp * 128:(cp + 1) * 128, :]
                        )
                        wo_t = pf_sb.tile([128, 256], BF16, tag="wot")
                        nc.sync.dma_start(wo_t[:], wo[:, k * 256:(k + 1) * 256])
                        st = dict(
                            start=(h == 0 and cp == 0), stop=(h == HPC - 1 and cp == 7)
                        )
                        for m in range(2):
                            lh = wo_t[:, m * 128:(m + 1) * 128]
                            for j in range(NT):
                                nc.tensor.matmul(
                                    ps_o[m][j][:], lh, yk[:, j * 512:(j + 1) * 512],
                                    **st,
                                )
                for m in range(2):
                    for j in range(NT):
                        ot = pf_sb.tile([128, 512], F32, tag="ot")
                        nc.vector.tensor_copy(ot[:], ps_o[m][j][:])
                        nc.sync.dma_start(
                            outT[m * 128:(m + 1) * 128, j * 512:(j + 1) * 512], ot[:]
                        )

        if bench_reps:
            with tc.For_i(0, bench_reps, 1):
                body()
        else:
            body()

    split_multiwaits(nc)
    return nc


# ---------------------------------------------------------------------------
# host side
# ---------------------------------------------------------------------------

_RUNNER_CACHE = None


def _make_runner(nc, n_cores=NCORES):
    """Build the sharded jit once; returns run(in_maps) -> list of out dicts."""
    import jax
    from jax.sharding import Mesh, NamedSharding, PartitionSpec
    from jax.experimental.shard_map import shard_map
    from concourse import bass2jax
    from concourse.bass2jax import _bass_exec_p, partition_id_tensor

    bass2jax.install_neuronx_cc_hook()

    partition_name = nc.partition_id_tensor.name if nc.partition_id_tensor else None
    in_names, out_names, out_avals, zero_outs = [], [], [], []
    for alloc in nc.m.functions[0].allocations:
        if not isinstance(alloc, mybir.MemoryLocationSet):
            continue
        name = alloc.memorylocations[0].name
        if alloc.kind == "ExternalInput":
            if name != partition_name:
                in_names.append(name)
        elif alloc.kind == "ExternalOutput":
            out_names.append(name)
            shape = tuple(alloc.tensor_shape)
            dtype = mybir.dt.np(alloc.dtype)
            out_avals.append(jax.core.ShapedArray(shape, dtype))
            zero_outs.append(np.zeros(shape, dtype))
    n_params = len(in_names)
    n_outs = len(out_avals)
    all_in_names = list(in_names) + list(out_names)
    if partition_name is not None:
        all_in_names.append(partition_name)
    donate = tuple(range(n_params, n_params + n_outs))

    def _body(*args):
        operands = list(args)
        if partition_name is not None:
            operands.append(partition_id_tensor())
        outs = _bass_exec_p.bind(
            *operands,
            out_avals=tuple(out_avals),
            in_names=tuple(all_in_names),
            out_names=tuple(out_names),
            lowering_input_output_aliases=(),
            sim_require_finite=True,
            sim_require_nnan=True,
            nc=nc,
        )
        return tuple(outs)

    devices = jax.devices()[:n_cores]
    mesh = Mesh(np.asarray(devices), ("core",))
    sharded = jax.jit(
        shard_map(
            _body, mesh=mesh,
            in_specs=(PartitionSpec("core"),) * (n_params + n_outs),
            out_specs=(PartitionSpec("core"),) * n_outs,
            check_rep=False,
        ),
        donate_argnums=donate,
        keep_unused=True,
    )
    shard = NamedSharding(mesh, PartitionSpec("core"))
    zshapes = [((n_cores * z.shape[0],) + z.shape[1:], z.dtype) for z in zero_outs]

    def run(in_maps):
        concat_in = [
            jax.device_put(
                np.concatenate(
                    [np.asarray(in_maps[c][n]) for c in range(n_cores)], axis=0
                ),
                shard,
            )
            for n in in_names
        ]
        zs = [jax.device_put(np.zeros(s, d), shard) for s, d in zshapes]
        outs = sharded(*concat_in, *zs)
        return [
            {
                name: np.asarray(outs[i]).reshape(n_cores, *out_avals[i].shape)[c]
                for i, name in enumerate(out_names)
            }
            for c in range(n_cores)
        ]

    return run


def _get_runner():
    global _RUNNER_CACHE
    if _RUNNER_CACHE is None:
        _RUNNER_CACHE = _make_runner(build_program())
    return _RUNNER_CACHE


def make_inputs(x, input_pos, Wq, Wk, Wv, Wo, q_norm_w, k_norm_w):
    """Host-side sharding / layout prep. Returns per-core input maps."""
    bf16 = _bf16()
    x2d = np.ascontiguousarray(np.asarray(x, np.float32).reshape(T, C)).astype(bf16)
    Wq = np.asarray(Wq, np.float32)
    Wk = np.asarray(Wk, np.float32)
    Wv = np.asarray(Wv, np.float32)
    Wo = np.asarray(Wo, np.float32)
    q_norm_w = np.asarray(q_norm_w, np.float32)
    k_norm_w = np.asarray(k_norm_w, np.float32)
    pos = np.asarray(input_pos, np.float32)

    # interleaved head-dim permutation: [0, 64, 1, 65, ...]
    perm = np.empty(128, np.int64)
    perm[0::2] = np.arange(64)
    perm[1::2] = np.arange(64) + 64

    # rope tables in interleaved layout (sign of the rotate-half folded in)
    inv_freq = (THETA ** (-(np.arange(0, D, 2, dtype=np.float32)) / D)).astype(
        np.float32
    )
    fr = pos[:, None] * inv_freq[None, :]  # [T, 64]
    cos = np.cos(fr).astype(np.float32).T  # [64, T]
    sin = np.sin(fr).astype(np.float32).T
    cos_il = np.empty((128, T), np.float32)
    cos_il[0::2] = cos
    cos_il[1::2] = cos
    sin_eff = np.empty((128, T), np.float32)
    sin_eff[0::2] = -sin
    sin_eff[1::2] = sin
    cos_il = np.ascontiguousarray(cos_il)
    sin_eff = np.ascontiguousarray(sin_eff)
    wqn_h = np.ascontiguousarray(q_norm_w[perm][None, :])
    wkn_h = np.ascontiguousarray(k_norm_w[perm][None, :])
    ident_h = np.eye(128, dtype=np.float32).astype(bf16)
    gg, pp = np.meshgrid(np.arange(896), np.arange(128))
    mask_h = (gg - pp - 384 >= 0).astype(np.float32).astype(bf16)

    Wq4 = Wq.reshape(N_HEAD, D, C)
    Wk4 = Wk.reshape(N_KV, D, C)
    Wv4 = Wv.reshape(N_KV, D, C)

    in_maps = []
    for c in range(NCORES):
        g = c // 2
        Wc = Wq4[HPC * c:HPC * (c + 1)][:, perm, :]  # [4, 128, C]
        wq_host = np.ascontiguousarray(
            Wc.reshape(HPC, 128, NK, 128).transpose(3, 0, 2, 1).reshape(128, -1)
        ).astype(bf16)
        wk_host = np.ascontiguousarray(
            Wk4[g][perm].reshape(128, NK, 128).transpose(2, 1, 0).reshape(128, -1)
        ).astype(bf16)
        wv_host = np.ascontiguousarray(
            Wv4[g].reshape(128, NK, 128).transpose(2, 1, 0).reshape(128, -1)
        ).astype(bf16)
        WoC = Wo[256 * c:256 * (c + 1), :]  # [256, 4096]
        wo_host = np.ascontiguousarray(
            WoC.reshape(2, 128, 32, 128).transpose(3, 2, 0, 1).reshape(128, -1)
        ).astype(bf16)
        in_maps.append(
            {
                "xb": x2d,
                "wq": wq_host,
                "wk": wk_host,
                "wv": wv_host,
                "wo": wo_host,
                "cost": cos_il,
                "sint": sin_eff,
                "wqn": wqn_h,
                "wkn": wkn_h,
                "identp": ident_h,
                "maskp": mask_h,
            }
        )
    return in_maps


def kernel(x, input_pos, Wq, Wk, Wv, Wo, q_norm_w, k_norm_w):
    run = _get_runner()
    in_maps = make_inputs(x, input_pos, Wq, Wk, Wv, Wo, q_norm_w, k_norm_w)
    results = run(in_maps)
    out = np.empty((1, T, C), np.float32)
    for c in range(NCORES):
        out[0][:, 256 * c:256 * (c + 1)] = results[c]["outT"].T
    return out



# revision 23
# speedup vs baseline: 2.5387x; 2.5387x over previous
"""Trainium2 Bass kernel for causal self-attention (GQA, RoPE, q/k-RMSNorm).

Sharding: tensor-parallel over heads across 8 cores.
  - core c owns q-heads [4c, 4c+4) and kv-head c//2 (each kv head serves 8 q heads)
  - x^T is built locally per core via XBAR DMA-transpose (bf16) into SBUF
  - attention is computed transposed (E^T = exp(K.Q^T)) so V in natural [S,D]
    layout is the matmul lhsT and y^T comes out in [D,T] layout directly
  - o_proj is computed LOCALLY per core: partial O^T = sum_h Wo[:,head h] @ y^T_h
    over the core's 4 heads, then ONE ReduceScatter per 512-column T-chunk
    sums partials across cores and leaves core c with O^T rows [256c,256c+256).
    The 4 chunked ReduceScatters overlap with the next chunk's attention.
  - the whole loop is j-chunked (T in 4 chunks of 512): QKV+norm+rope ->
    attention (4 heads) -> o_proj partial -> ReduceScatter, all per chunk
  - head-dim rows of q/k are interleaved (d -> [0,64,1,65,...]) so the RoPE
    rotate-half becomes an adjacent-pair partition swap (one stream_shuffle);
    the rmsnorm weight is folded into the host-precomputed cos/sin tables;
    the 1/rms and 1/den broadcasts over partitions use rank-1 PE matmuls
  - everything flows in bf16 (f32 PSUM accumulation), incl. the output

Engines: PE does all matmuls; Act does PSUM->SBUF copies + exp/rsqrt/recip;
DVE does the rope elementwise chain + final scaling; collectives on gpsimd.
"""

import sys

sys.path.insert(0, "/opt/trn_rl_repo")

from contextlib import ExitStack

import numpy as np

import bass_rust
import concourse.bass as bass
import concourse.mybir as mybir
from concourse import tile

F32 = mybir.dt.float32
BF16 = mybir.dt.bfloat16

N_HEAD = 32
N_KV = 4
D = 128
C = 2048
T = 2048
NCORES = 8
HPC = N_HEAD // NCORES  # q heads per core = 4
THETA = 1000000.0
EPS = 1e-6
SCALE = 1.0 / np.sqrt(128.0)

NT = T // 512  # 4 T-chunks of 512
NK = C // 128  # 16 contraction tiles for qkv
NS = T // 128  # 16 S-blocks of 128
NM = C // 128  # 16 output m-tiles for o_proj

# stream_shuffle swaps within each 32-partition quadrant; adjacent-pair swap
SWAP_MASK = [i ^ 1 for i in range(32)]

_BF16_NP = None


def _bf16():
    global _BF16_NP
    if _BF16_NP is None:
        import ml_dtypes

        _BF16_NP = np.dtype(ml_dtypes.bfloat16)
    return _BF16_NP


def split_multiwaits(nc):
    """The walrus build in this container supports one sync-wait per
    instruction; hoist extra waits onto NOPs inserted before the offender."""
    ctr = 0
    for f in nc.m.functions:
        for bb in f.blocks:
            new_insts = []
            changed = False
            for inst in bb.instructions:
                si = inst.sync_info
                if si is not None and si.on_wait and len(si.on_wait) > 1:
                    waits = list(si.on_wait)
                    for w in waits[:-1]:
                        ctr += 1
                        nop = bass_rust.InstNoOp(name=f"splitw-{ctr}", ins=[], outs=[])
                        nop.engine = inst.engine
                        nop.sync_info = bass_rust.SyncInfo(on_wait=[w], on_update=[])
                        new_insts.append(nop)
                    inst.sync_info = bass_rust.SyncInfo(
                        on_wait=[waits[-1]], on_update=list(si.on_update or [])
                    )
                    changed = True
                new_insts.append(inst)
            if changed:
                bb.instructions = new_insts


def build_program(bench_reps=0, phases="ABDF"):
    nc = bass.Bass("TRN2", target_bir_lowering=False, debug=False, num_devices=NCORES)

    xb = nc.declare_dram_parameter("xb", [T, C], BF16, isOutput=False)
    wq = nc.declare_dram_parameter("wq", [128, HPC * NK * 128], BF16, isOutput=False)
    wk = nc.declare_dram_parameter("wk", [128, NK * 128], BF16, isOutput=False)
    wv = nc.declare_dram_parameter("wv", [128, NK * 128], BF16, isOutput=False)
    wo = nc.declare_dram_parameter("wo", [128, NM * HPC * 128], BF16, isOutput=False)
    wqc = nc.declare_dram_parameter("wqc", [128, T], BF16, isOutput=False)
    wqs = nc.declare_dram_parameter("wqs", [128, T], BF16, isOutput=False)
    wkc = nc.declare_dram_parameter("wkc", [128, T], BF16, isOutput=False)
    wks = nc.declare_dram_parameter("wks", [128, T], BF16, isOutput=False)
    identp = nc.declare_dram_parameter("identp", [128, 128], BF16, isOutput=False)
    mnegp = nc.declare_dram_parameter("mnegp", [128, 128], BF16, isOutput=False)
    rs_out = [
        nc.declare_dram_parameter(f"rs{j}", [2 * 128, 512], BF16, isOutput=True)
        for j in range(NT)
    ]

    rg = [list(range(NCORES))]
    collectives = bench_reps == 0

    with tile.TileContext(nc) as tc, ExitStack() as ctx:
        const = ctx.enter_context(tc.tile_pool(name="const", bufs=1))
        wpool = ctx.enter_context(tc.tile_pool(name="wpool", bufs=1))
        act = ctx.enter_context(tc.tile_pool(name="act", bufs=1))
        work = ctx.enter_context(tc.tile_pool(name="work", bufs=2))
        etp = ctx.enter_context(tc.tile_pool(name="etp", bufs=6))
        P = ctx.enter_context(tc.tile_pool(name="P", bufs=1, space="PSUM"))
        dram = ctx.enter_context(tc.tile_pool(name="dram", bufs=1, space="DRAM"))

        # ---- constants ----
        ones_colb = const.tile([128, 1], BF16)
        nc.vector.memset(ones_colb[:], 1.0)
        ones_rowb = const.tile([1, 128], BF16)
        nc.vector.memset(ones_rowb[:], 1.0)
        eps_col = const.tile([128, 1], F32)
        nc.vector.memset(eps_col[:], EPS)
        mnegb = const.tile([128, 128], BF16)
        # [I | 0]: rhs for the PE causal-mask add (cols >= 128 contribute 0)
        idz = const.tile([128, 512], BF16)
        nc.vector.memset(idz[:], 0.0)

        # ---- resident weights / tables / activations ----
        wq_sb = wpool.tile([128, HPC * NK * 128], BF16)
        wk_sb = wpool.tile([128, NK * 128], BF16)
        wv_sb = wpool.tile([128, NK * 128], BF16)
        wo_sb = wpool.tile([128, NM * HPC * 128], BF16)
        tqc = wpool.tile([128, T], BF16)
        tqs = wpool.tile([128, T], BF16)
        tkc = wpool.tile([128, T], BF16)
        tks = wpool.tile([128, T], BF16)

        xT = [act.tile([128, T], BF16, name=f"xT{k}") for k in range(NK)]
        qT = [act.tile([128, T], BF16, name=f"qT{h}") for h in range(HPC)]
        kT = act.tile([128, T], BF16)
        vN = act.tile([128, NS * 128], BF16)  # natural [S,D] as 16 s-tiles
        yTj = [act.tile([128, 512], BF16, name=f"yTj{h}") for h in range(HPC)]

        part = [dram.tile([C, 512], BF16, name=f"part{j}") for j in range(NT)]
        rsb = [dram.tile([2 * 128, 512], BF16, name=f"rsb{j}") for j in range(NT)]

        # upfront loads, ordered by first use (nc.sync queue, in-order).
        # xT for chunks j>0 is prefetched inside the j-loop (one chunk ahead)
        # so the per-chunk vN transposes don't queue behind them.
        nc.sync.dma_start(wk_sb[:], wk[:, :])
        nc.sync.dma_start(wv_sb[:], wv[:, :])
        if "A" in phases:
            for k in range(NK):
                js = slice(0, 512)
                nc.sync.dma_start(
                    xT[k][:, js], xb[js, k * 128:(k + 1) * 128], transpose=True
                )
        nc.sync.dma_start(wq_sb[:], wq[:, :])
        nc.sync.dma_start(tkc[:], wkc[:, :])
        nc.sync.dma_start(tks[:], wks[:, :])
        nc.sync.dma_start(tqc[:], wqc[:, :])
        nc.sync.dma_start(tqs[:], wqs[:, :])
        nc.sync.dma_start(wo_sb[:], wo[:, :])
        nc.sync.dma_start(mnegb[:], mnegp[:, :])
        nc.sync.dma_start(idz[:, 0:128], identp[:, :])

        def norm_rope(ps, wcos, wsin, dest, js):
            """dest[:, js] = rope(rmsnorm-scaled ps); weight folded in tables."""
            raw = work.tile([128, 512], BF16, tag="raw")
            nc.scalar.activation(
                raw[:], ps[:], mybir.ActivationFunctionType.Copy, scale=1.0
            )
            sqr = work.tile([128, 512], BF16, tag="sqr")
            nc.vector.tensor_mul(sqr[:], raw[:], raw[:])
            ssq = P.tile([128, 512], F32, tag="p6")
            nc.tensor.matmul(ssq[0:1, :], ones_colb[:], sqr[:])
            rms = work.tile([1, 512], F32, tag="rms")
            nc.scalar.activation(
                rms[:], ssq[0:1, :], mybir.ActivationFunctionType.Sqrt,
                scale=1.0 / 128.0, bias=eps_col[0:1, :],
            )
            rinv = work.tile([1, 512], BF16, tag="rinv")
            with nc.allow_low_precision(reason="feeds PE broadcast"):
                nc.vector.reciprocal(rinv[:], rms[:])
            rb = P.tile([128, 512], F32, tag="p7")
            nc.tensor.matmul(rb[:], ones_rowb[:], rinv[:])
            qs = work.tile([128, 512], BF16, tag="qs")
            nc.vector.stream_shuffle(qs[:], raw[:], mask=SWAP_MASK)
            t1 = work.tile([128, 512], BF16, tag="t1")
            nc.vector.tensor_mul(t1[:], raw[:], wcos[:, js])
            t2 = work.tile([128, 512], BF16, tag="t2")
            nc.vector.tensor_mul(t2[:], qs[:], wsin[:, js])
            pre = work.tile([128, 512], BF16, tag="pre")
            nc.vector.tensor_add(pre[:], t1[:], t2[:])
            nc.vector.tensor_mul(dest[:, js], pre[:], rb[:])

        def body():
            for j in range(NT):
                js = slice(j * 512, (j + 1) * 512)

                # ===== QKV projection + RMSNorm + RoPE for chunk j =====
                # K and V first so kT/vN are ready when attention starts.
                if "B" in phases:
                    ps_k = P.tile([128, 512], F32, tag="p4")
                    for k in range(NK):
                        nc.tensor.matmul(
                            ps_k[:], wk_sb[:, k * 128:(k + 1) * 128], xT[k][:, js],
                            start=(k == 0), stop=(k == NK - 1),
                        )
                    norm_rope(ps_k, tkc, tks, kT, js)
                    ps_v = P.tile([128, 512], F32, tag="p5")
                    for k in range(NK):
                        nc.tensor.matmul(
                            ps_v[:], wv_sb[:, k * 128:(k + 1) * 128], xT[k][:, js],
                            start=(k == 0), stop=(k == NK - 1),
                        )
                    # v: [D, T]-chunk -> natural [S, D] tiles via XBAR transpose
                    vt = work.tile([128, 512], BF16, tag="vt")
                    nc.scalar.activation(
                        vt[:], ps_v[:], mybir.ActivationFunctionType.Copy, scale=1.0
                    )
                    for u in range(4):
                        s_tile = j * 4 + u
                        nc.sync.dma_start(
                            vN[:, s_tile * 128:(s_tile + 1) * 128],
                            vt[:, u * 128:(u + 1) * 128], transpose=True,
                        )
                    if "A" in phases and j == 0:
                        rjs = slice(512, T)
                        for k in range(NK):
                            nc.sync.dma_start(
                                xT[k][:, rjs], xb[rjs, k * 128:(k + 1) * 128],
                                transpose=True,
                            )
                    ps_q = [
                        P.tile([128, 512], F32, tag=f"p{h}", name=f"psq{h}")
                        for h in range(HPC)
                    ]
                    for h in range(HPC):
                        for k in range(NK):
                            nc.tensor.matmul(
                                ps_q[h][:],
                                wq_sb[:, (h * NK + k) * 128:(h * NK + k + 1) * 128],
                                xT[k][:, js], start=(k == 0), stop=(k == NK - 1),
                            )
                        norm_rope(ps_q[h], tqc, tqs, qT[h], js)

                # ===== attention for chunk j, all 4 local heads =====
                if "D" not in phases:
                    continue
                nblk = 4 * j + 4
                LOOK = 3
                SROT = (0, 1, 5, 6)  # rotating PSUM banks for score lookahead
                for h in range(HPC):
                    ps_y = P.tile([128, 512], F32, tag="p2")
                    ps_den = P.tile([128, 512], F32, tag="p3")

                    def emit_score(i):
                        u = i - 4 * j
                        fs = 0 if u < 0 else 128 * u
                        ps_s = P.tile(
                            [128, 512], F32, tag=f"p{SROT[i % 4]}",
                            name=f"pss{SROT[i % 4]}",
                        )
                        qslice = qT[h][:, j * 512 + fs:(j + 1) * 512]
                        if u >= 0:
                            # diagonal: add -BIG upper-triangle via PE so exp -> 0
                            nc.tensor.matmul(
                                ps_s[:, fs:512], kT[:, i * 128:(i + 1) * 128],
                                qslice, start=True, stop=False,
                            )
                            nc.tensor.matmul(
                                ps_s[:, fs:fs + 128], mnegb[:], idz[:, 0:128],
                                start=False, stop=True,
                            )
                        else:
                            nc.tensor.matmul(
                                ps_s[:, fs:512], kT[:, i * 128:(i + 1) * 128], qslice
                            )
                        et = etp.tile([128, 512], BF16, tag="et")
                        nc.scalar.activation(
                            et[:, fs:512], ps_s[:, fs:512],
                            mybir.ActivationFunctionType.Exp, scale=float(SCALE),
                        )
                        return et, fs

                    pend = {}
                    for i in range(min(LOOK, nblk)):
                        pend[i] = emit_score(i)
                    for i in range(nblk):
                        if i + LOOK < nblk:
                            pend[i + LOOK] = emit_score(i + LOOK)
                        et, fs = pend.pop(i)
                        st = dict(start=(i == 0), stop=(i == nblk - 1))
                        nc.tensor.matmul(
                            ps_y[:, fs:512], vN[:, i * 128:(i + 1) * 128],
                            et[:, fs:512], **st,
                        )
                        nc.tensor.matmul(
                            ps_den[0:1, fs:512], ones_colb[:], et[:, fs:512], **st
                        )
                    rd = work.tile([1, 512], BF16, tag="rd")
                    with nc.allow_low_precision(reason="feeds PE broadcast"):
                        nc.vector.reciprocal(rd[:], ps_den[0:1, :])
                    ps_rb = P.tile([128, 512], F32, tag="p4")
                    nc.tensor.matmul(ps_rb[:], ones_rowb[:], rd[:])
                    yb = work.tile([128, 512], BF16, tag="yb")
                    nc.scalar.activation(
                        yb[:], ps_y[:], mybir.ActivationFunctionType.Copy, scale=1.0
                    )
                    nc.vector.tensor_mul(yTj[h][:], yb[:], ps_rb[:])

                # ===== o_proj partial for chunk j + ReduceScatter =====
                if "F" not in phases:
                    continue
                for mp in range(NM // 2):
                    ob2 = work.tile([128, 1024], BF16, tag=f"ob{mp % 3}")
                    for half in range(2):
                        m = 2 * mp + half
                        ps_o = P.tile([128, 512], F32, tag=f"p{5 + m % 3}")
                        for h in range(HPC):
                            nc.tensor.matmul(
                                ps_o[:],
                                wo_sb[:, (m * HPC + h) * 128:(m * HPC + h + 1) * 128],
                                yTj[h][:], start=(h == 0), stop=(h == HPC - 1),
                            )
                        dst = ob2[:, half * 512:(half + 1) * 512]
                        nc.vector.tensor_copy(dst, ps_o[:])
                    nc.sync.dma_start(
                        part[j][2 * mp * 128:(2 * mp + 2) * 128, :].rearrange(
                            "(two r) c -> r two c", two=2
                        ),
                        ob2[:],
                    )
                if collectives:
                    nc.gpsimd.collective_compute(
                        "ReduceScatter", mybir.AluOpType.add, replica_groups=rg,
                        ins=[part[j][:].opt()], outs=[rsb[j][:].opt()],
                    )
                    nc.sync.dma_start(rs_out[j][:, :], rsb[j][:])

        if bench_reps:
            with tc.For_i(0, bench_reps, 1):
                body()
        else:
            body()

    split_multiwaits(nc)
    return nc


# ---------------------------------------------------------------------------
# host side
# ---------------------------------------------------------------------------

_RUNNER_CACHE = None


def _make_runner(nc, n_cores=NCORES):
    """Build the sharded jit once; returns run(in_maps) -> list of out dicts."""
    import jax
    from jax.sharding import Mesh, NamedSharding, PartitionSpec
    from jax.experimental.shard_map import shard_map
    from concourse import bass2jax
    from concourse.bass2jax import _bass_exec_p, partition_id_tensor

    bass2jax.install_neuronx_cc_hook()

    partition_name = nc.partition_id_tensor.name if nc.partition_id_tensor else None
    in_names, out_names, out_avals, zero_outs = [], [], [], []
    for alloc in nc.m.functions[0].allocations:
        if not isinstance(alloc, mybir.MemoryLocationSet):
            continue
        name = alloc.memorylocations[0].name
        if alloc.kind == "ExternalInput":
            if name != partition_name:
                in_names.append(name)
        elif alloc.kind == "ExternalOutput":
            out_names.append(name)
            shape = tuple(alloc.tensor_shape)
            dtype = mybir.dt.np(alloc.dtype)
            out_avals.append(jax.core.ShapedArray(shape, dtype))
            zero_outs.append(np.zeros(shape, dtype))
    n_params = len(in_names)
    n_outs = len(out_avals)
    all_in_names = list(in_names) + list(out_names)
    if partition_name is not None:
        all_in_names.append(partition_name)
    donate = tuple(range(n_params, n_params + n_outs))

    def _body(*args):
        operands = list(args)
        if partition_name is not None:
            operands.append(partition_id_tensor())
        outs = _bass_exec_p.bind(
            *operands,
            out_avals=tuple(out_avals),
            in_names=tuple(all_in_names),
            out_names=tuple(out_names),
            lowering_input_output_aliases=(),
            sim_require_finite=True,
            sim_require_nnan=True,
            nc=nc,
        )
        return tuple(outs)

    devices = jax.devices()[:n_cores]
    mesh = Mesh(np.asarray(devices), ("core",))
    sharded = jax.jit(
        shard_map(
            _body, mesh=mesh,
            in_specs=(PartitionSpec("core"),) * (n_params + n_outs),
            out_specs=(PartitionSpec("core"),) * n_outs,
            check_rep=False,
        ),
        donate_argnums=donate,
        keep_unused=True,
    )
    shard = NamedSharding(mesh, PartitionSpec("core"))
    zshapes = [((n_cores * z.shape[0],) + z.shape[1:], z.dtype) for z in zero_outs]

    def run(in_maps):
        concat_in = [
            jax.device_put(
                np.concatenate(
                    [np.asarray(in_maps[c][n]) for c in range(n_cores)], axis=0
                ),
                shard,
            )
            for n in in_names
        ]
        zs = [jax.device_put(np.zeros(s, d), shard) for s, d in zshapes]
        outs = sharded(*concat_in, *zs)
        return [
            {
                name: np.asarray(outs[i]).reshape(n_cores, *out_avals[i].shape)[c]
                for i, name in enumerate(out_names)
            }
            for c in range(n_cores)
        ]

    return run


def _get_runner():
    global _RUNNER_CACHE
    if _RUNNER_CACHE is None:
        _RUNNER_CACHE = _make_runner(build_program())
    return _RUNNER_CACHE


def make_inputs(x, input_pos, Wq, Wk, Wv, Wo, q_norm_w, k_norm_w):
    """Host-side sharding / layout prep. Returns per-core input maps."""
    bf16 = _bf16()
    x2d = np.ascontiguousarray(np.asarray(x, np.float32).reshape(T, C)).astype(bf16)
    Wq = np.asarray(Wq, np.float32)
    Wk = np.asarray(Wk, np.float32)
    Wv = np.asarray(Wv, np.float32)
    Wo = np.asarray(Wo, np.float32)
    q_norm_w = np.asarray(q_norm_w, np.float32)
    k_norm_w = np.asarray(k_norm_w, np.float32)
    pos = np.asarray(input_pos, np.float32)

    # interleaved head-dim permutation: [0, 64, 1, 65, ...]
    perm = np.empty(128, np.int64)
    perm[0::2] = np.arange(64)
    perm[1::2] = np.arange(64) + 64
    pswap = np.arange(128) ^ 1  # adjacent-pair swap of interleaved rows

    # rope tables in interleaved layout (sign of the rotate-half folded in),
    # with the rmsnorm weight folded in: the even/odd rows of the sin table
    # carry the weight of the PAIRED row (the shuffled operand).
    inv_freq = (THETA ** (-(np.arange(0, D, 2, dtype=np.float32)) / D)).astype(
        np.float32
    )
    fr = pos[:, None] * inv_freq[None, :]  # [T, 64]
    cos = np.cos(fr).astype(np.float32).T  # [64, T]
    sin = np.sin(fr).astype(np.float32).T
    cos_il = np.empty((128, T), np.float32)
    cos_il[0::2] = cos
    cos_il[1::2] = cos
    sin_eff = np.empty((128, T), np.float32)
    sin_eff[0::2] = -sin
    sin_eff[1::2] = sin

    def fold(w):
        wp = w[perm]
        wc = np.ascontiguousarray(cos_il * wp[:, None]).astype(bf16)
        ws = np.ascontiguousarray(sin_eff * wp[pswap][:, None]).astype(bf16)
        return wc, ws

    wqc_h, wqs_h = fold(q_norm_w)
    wkc_h, wks_h = fold(k_norm_w)

    ident_h = np.eye(128, dtype=np.float32).astype(bf16)
    # mneg[r, c] = -BIG iff r < c; with rhs=[I|0] this adds -BIG to score[p, x]
    # for x < p (future positions) so exp underflows to exactly 0.
    rr, cc2 = np.meshgrid(np.arange(128), np.arange(128), indexing="ij")
    mneg_h = np.where(rr < cc2, np.float32(-1e30), np.float32(0)).astype(bf16)

    Wq4 = Wq.reshape(N_HEAD, D, C)
    Wk4 = Wk.reshape(N_KV, D, C)
    Wv4 = Wv.reshape(N_KV, D, C)

    in_maps = []
    for c in range(NCORES):
        g = c // 2
        Wc = Wq4[HPC * c:HPC * (c + 1)][:, perm, :]  # [4, 128, C]
        wq_host = np.ascontiguousarray(
            Wc.reshape(HPC, 128, NK, 128).transpose(3, 0, 2, 1).reshape(128, -1)
        ).astype(bf16)
        wk_host = np.ascontiguousarray(
            Wk4[g][perm].reshape(128, NK, 128).transpose(2, 1, 0).reshape(128, -1)
        ).astype(bf16)
        wv_host = np.ascontiguousarray(
            Wv4[g].reshape(128, NK, 128).transpose(2, 1, 0).reshape(128, -1)
        ).astype(bf16)
        # o_proj lhsT per (m-tile, local head): wo_host[d, (m*4+h)*128 + i]
        # = Wo[128m+i, 512c + 128h + d]
        WoC = Wo[:, 512 * c:512 * (c + 1)]  # [2048, 512]
        wo_host = np.ascontiguousarray(
            WoC.reshape(NM, 128, HPC, 128).transpose(3, 0, 2, 1).reshape(128, -1)
        ).astype(bf16)
        in_maps.append(
            {
                "xb": x2d,
                "wq": wq_host,
                "wk": wk_host,
                "wv": wv_host,
                "wo": wo_host,
                "wqc": wqc_h,
                "wqs": wqs_h,
                "wkc": wkc_h,
                "wks": wks_h,
                "identp": ident_h,
                "mnegp": mneg_h,
            }
        )
    return in_maps


def kernel(x, input_pos, Wq, Wk, Wv, Wo, q_norm_w, k_norm_w):
    run = _get_runner()
    in_maps = make_inputs(x, input_pos, Wq, Wk, Wv, Wo, q_norm_w, k_norm_w)
    results = run(in_maps)
    out = np.empty((1, T, C), np.float32)
    for c in range(NCORES):
        for j in range(NT):
            out[0][j * 512:(j + 1) * 512, 256 * c:256 * (c + 1)] = (
                results[c][f"rs{j}"].astype(np.float32).T
            )
    return out


# revision 34
# speedup vs baseline: 2.6213x; 1.0325x over previous
"""Trainium2 Bass kernel for causal self-attention (GQA, RoPE, q/k-RMSNorm).

Sharding: tensor-parallel over heads across 8 cores.
  - core c owns q-heads [4c, 4c+4) and kv-head c//2 (each kv head serves 8 q heads)
  - x^T is built locally per core via XBAR DMA-transpose (bf16) into SBUF
  - attention is computed transposed (E^T = exp(K.Q^T)) so V in natural [S,D]
    layout is the matmul lhsT and y^T comes out in [D,T] layout directly
  - o_proj is computed LOCALLY per core: partial O^T = sum_h Wo[:,head h] @ y^T_h
    over the core's 4 heads, then ONE ReduceScatter per 512-column T-chunk
    sums partials across cores and leaves core c with O^T rows [256c,256c+256).
    The 4 chunked ReduceScatters overlap with the next chunk's attention.
  - the whole loop is j-chunked (T in 4 chunks of 512): QKV+norm+rope ->
    attention (4 heads) -> o_proj partial -> ReduceScatter, all per chunk
  - head-dim rows of q/k are interleaved (d -> [0,64,1,65,...]) so the RoPE
    rotate-half becomes an adjacent-pair partition swap (one stream_shuffle);
    the rmsnorm weight is folded into the host-precomputed cos/sin tables;
    the 1/rms and 1/den broadcasts over partitions use rank-1 PE matmuls
  - everything flows in bf16 (f32 PSUM accumulation), incl. the output

Engines: PE does all matmuls; Act does PSUM->SBUF copies + exp/rsqrt/recip;
DVE does the rope elementwise chain + final scaling; collectives on gpsimd.
"""

import sys

sys.path.insert(0, "/opt/trn_rl_repo")

from contextlib import ExitStack

import numpy as np

import bass_rust
import concourse.bass as bass
import concourse.mybir as mybir
from concourse import tile

F32 = mybir.dt.float32
BF16 = mybir.dt.bfloat16

N_HEAD = 32
N_KV = 4
D = 128
C = 2048
T = 2048
NCORES = 8
HPC = N_HEAD // NCORES  # q heads per core = 4
THETA = 1000000.0
EPS = 1e-6
SCALE = 1.0 / np.sqrt(128.0)

NT = T // 512  # 4 T-chunks of 512
NK = C // 128  # 16 contraction tiles for qkv
NS = T // 128  # 16 S-blocks of 128
NM = C // 128  # 16 output m-tiles for o_proj

# stream_shuffle swaps within each 32-partition quadrant; adjacent-pair swap
SWAP_MASK = [i ^ 1 for i in range(32)]

_BF16_NP = None


def _bf16():
    global _BF16_NP
    if _BF16_NP is None:
        import ml_dtypes

        _BF16_NP = np.dtype(ml_dtypes.bfloat16)
    return _BF16_NP


def split_multiwaits(nc):
    """The walrus build in this container supports one sync-wait per
    instruction; hoist extra waits onto NOPs inserted before the offender."""
    ctr = 0
    for f in nc.m.functions:
        for bb in f.blocks:
            new_insts = []
            changed = False
            for inst in bb.instructions:
                si = inst.sync_info
                if si is not None and si.on_wait and len(si.on_wait) > 1:
                    waits = list(si.on_wait)
                    for w in waits[:-1]:
                        ctr += 1
                        nop = bass_rust.InstNoOp(name=f"splitw-{ctr}", ins=[], outs=[])
                        nop.engine = inst.engine
                        nop.sync_info = bass_rust.SyncInfo(on_wait=[w], on_update=[])
                        new_insts.append(nop)
                    inst.sync_info = bass_rust.SyncInfo(
                        on_wait=[waits[-1]], on_update=list(si.on_update or [])
                    )
                    changed = True
                new_insts.append(inst)
            if changed:
                bb.instructions = new_insts


def build_program(bench_reps=0, phases="ABDF"):
    nc = bass.Bass("TRN2", target_bir_lowering=False, debug=False, num_devices=NCORES)

    xb = nc.declare_dram_parameter("xb", [T, C], BF16, isOutput=False)
    wq = nc.declare_dram_parameter("wq", [128, HPC * NK * 128], BF16, isOutput=False)
    wk = nc.declare_dram_parameter("wk", [128, NK * 128], BF16, isOutput=False)
    wv = nc.declare_dram_parameter("wv", [128, NK * 128], BF16, isOutput=False)
    wo = nc.declare_dram_parameter("wo", [128, NM * HPC * 128], BF16, isOutput=False)
    wqc = nc.declare_dram_parameter("wqc", [128, T], BF16, isOutput=False)
    wqs = nc.declare_dram_parameter("wqs", [128, T], BF16, isOutput=False)
    wkc = nc.declare_dram_parameter("wkc", [128, T], BF16, isOutput=False)
    wks = nc.declare_dram_parameter("wks", [128, T], BF16, isOutput=False)
    identp = nc.declare_dram_parameter("identp", [128, 128], BF16, isOutput=False)
    mnegp = nc.declare_dram_parameter("mnegp", [128, 128], BF16, isOutput=False)
    rs_out = [
        nc.declare_dram_parameter(f"rs{j}", [2 * 128, 512], BF16, isOutput=True)
        for j in range(NT)
    ]

    rg = [list(range(NCORES))]
    collectives = bench_reps == 0

    with tile.TileContext(nc) as tc, ExitStack() as ctx:
        const = ctx.enter_context(tc.tile_pool(name="const", bufs=1))
        wpool = ctx.enter_context(tc.tile_pool(name="wpool", bufs=1))
        act = ctx.enter_context(tc.tile_pool(name="act", bufs=1))
        work = ctx.enter_context(tc.tile_pool(name="work", bufs=2))
        etp = ctx.enter_context(tc.tile_pool(name="etp", bufs=6))
        P = ctx.enter_context(tc.tile_pool(name="P", bufs=1, space="PSUM"))
        dram = ctx.enter_context(tc.tile_pool(name="dram", bufs=1, space="DRAM"))

        # ---- constants ----
        ones_colb = const.tile([128, 1], BF16)
        nc.vector.memset(ones_colb[:], 1.0)
        ones_rowb = const.tile([1, 128], BF16)
        nc.vector.memset(ones_rowb[:], 1.0)
        eps_col = const.tile([128, 1], F32)
        nc.vector.memset(eps_col[:], EPS)
        mnegb = const.tile([128, 128], BF16)
        # [I | 0]: rhs for the PE causal-mask add (cols >= 128 contribute 0)
        idz = const.tile([128, 512], BF16)
        nc.vector.memset(idz[:], 0.0)

        # ---- resident weights / tables / activations ----
        wq_sb = wpool.tile([128, HPC * NK * 128], BF16)
        wk_sb = wpool.tile([128, NK * 128], BF16)
        wv_sb = wpool.tile([128, NK * 128], BF16)
        wo_sb = wpool.tile([128, NM * HPC * 128], BF16)
        tqc = wpool.tile([128, T], BF16)
        tqs = wpool.tile([128, T], BF16)
        tkc = wpool.tile([128, T], BF16)
        tks = wpool.tile([128, T], BF16)

        xT = [act.tile([128, T], BF16, name=f"xT{k}") for k in range(NK)]
        qT = [act.tile([128, T], BF16, name=f"qT{h}") for h in range(HPC)]
        kT = act.tile([128, T], BF16)
        vN = act.tile([128, NS * 128], BF16)  # natural [S,D] as 16 s-tiles
        yTj = [act.tile([128, 512], BF16, name=f"yTj{h}") for h in range(HPC)]

        part = [dram.tile([C, 512], BF16, name=f"part{j}") for j in range(NT)]
        rsb = [dram.tile([2 * 128, 512], BF16, name=f"rsb{j}") for j in range(NT)]

        # upfront loads, ordered by first use (nc.sync queue, in-order).
        # xT for chunks j>0 is prefetched inside the j-loop (one chunk ahead)
        # so the per-chunk vN transposes don't queue behind them.
        nc.sync.dma_start(wk_sb[:], wk[:, :])
        nc.sync.dma_start(wv_sb[:], wv[:, :])
        nc.sync.dma_start(wq_sb[:], wq[:, :])
        if "A" in phases:
            for k in range(NK):
                js = slice(0, 512)
                nc.sync.dma_start(
                    xT[k][:, js], xb[js, k * 128:(k + 1) * 128], transpose=True
                )
        nc.sync.dma_start(tkc[:], wkc[:, :])
        nc.sync.dma_start(tks[:], wks[:, :])
        nc.sync.dma_start(tqc[:], wqc[:, :])
        nc.sync.dma_start(tqs[:], wqs[:, :])
        nc.sync.dma_start(wo_sb[:], wo[:, :])
        nc.sync.dma_start(mnegb[:], mnegp[:, :])
        nc.sync.dma_start(idz[:, 0:128], identp[:, :])

        def norm_rope(ps, wcos, wsin, dest, js):
            """dest[:, js] = rope(rmsnorm-scaled ps); weight folded in tables."""
            raw = work.tile([128, 512], BF16, tag="raw")
            nc.scalar.activation(
                raw[:], ps[:], mybir.ActivationFunctionType.Copy, scale=1.0
            )
            sqr = work.tile([128, 512], BF16, tag="sqr")
            nc.vector.tensor_mul(sqr[:], raw[:], raw[:])
            ssq = P.tile([128, 512], F32, tag="p6")
            nc.tensor.matmul(ssq[0:1, :], ones_colb[:], sqr[:])
            rms = work.tile([1, 512], F32, tag="rms")
            nc.scalar.activation(
                rms[:], ssq[0:1, :], mybir.ActivationFunctionType.Sqrt,
                scale=1.0 / 128.0, bias=eps_col[0:1, :],
            )
            rinv = work.tile([1, 512], BF16, tag="rinv")
            with nc.allow_low_precision(reason="feeds PE broadcast"):
                nc.vector.reciprocal(rinv[:], rms[:])
            rb = P.tile([128, 512], F32, tag="p7")
            nc.tensor.matmul(rb[:], ones_rowb[:], rinv[:])
            qs = work.tile([128, 512], BF16, tag="qs")
            nc.vector.stream_shuffle(qs[:], raw[:], mask=SWAP_MASK)
            t1 = work.tile([128, 512], BF16, tag="t1")
            nc.vector.tensor_mul(t1[:], raw[:], wcos[:, js])
            t2 = work.tile([128, 512], BF16, tag="t2")
            nc.vector.tensor_mul(t2[:], qs[:], wsin[:, js])
            pre = work.tile([128, 512], BF16, tag="pre")
            nc.vector.tensor_add(pre[:], t1[:], t2[:])
            nc.vector.tensor_mul(dest[:, js], pre[:], rb[:])

        def body():
            for j in range(NT):
                js = slice(j * 512, (j + 1) * 512)

                # ===== QKV projection + RMSNorm + RoPE for chunk j =====
                # K and V first so kT/vN are ready when attention starts.
                if "B" in phases:
                    ps_v = P.tile([128, 512], F32, tag="p5")
                    for k in range(NK):
                        nc.tensor.matmul(
                            ps_v[:], wv_sb[:, k * 128:(k + 1) * 128], xT[k][:, js],
                            start=(k == 0), stop=(k == NK - 1),
                        )
                    # v: [D, T]-chunk -> natural [S, D] tiles via XBAR transpose
                    vt = work.tile([128, 512], BF16, tag="vt")
                    nc.scalar.activation(
                        vt[:], ps_v[:], mybir.ActivationFunctionType.Copy, scale=1.0
                    )
                    for u in range(4):
                        s_tile = j * 4 + u
                        nc.sync.dma_start(
                            vN[:, s_tile * 128:(s_tile + 1) * 128],
                            vt[:, u * 128:(u + 1) * 128], transpose=True,
                        )
                    if "A" in phases and j == 0:
                        rjs = slice(512, T)
                        for k in range(NK):
                            nc.sync.dma_start(
                                xT[k][:, rjs], xb[rjs, k * 128:(k + 1) * 128],
                                transpose=True,
                            )
                    ps_k = P.tile([128, 512], F32, tag="p4")
                    for k in range(NK):
                        nc.tensor.matmul(
                            ps_k[:], wk_sb[:, k * 128:(k + 1) * 128], xT[k][:, js],
                            start=(k == 0), stop=(k == NK - 1),
                        )
                    norm_rope(ps_k, tkc, tks, kT, js)
                    ps_q = [
                        P.tile([128, 512], F32, tag=f"p{h}", name=f"psq{h}")
                        for h in range(HPC)
                    ]
                    for h in range(HPC):
                        for k in range(NK):
                            nc.tensor.matmul(
                                ps_q[h][:],
                                wq_sb[:, (h * NK + k) * 128:(h * NK + k + 1) * 128],
                                xT[k][:, js], start=(k == 0), stop=(k == NK - 1),
                            )
                        norm_rope(ps_q[h], tqc, tqs, qT[h], js)

                # ===== attention for chunk j, all 4 local heads =====
                if "D" not in phases:
                    continue
                nblk = 4 * j + 4
                LOOK = 3
                SROT = (0, 1, 5, 6)  # rotating PSUM banks for score lookahead
                for h in range(HPC):
                    ps_y = P.tile([128, 512], F32, tag="p2")
                    ps_den = P.tile([128, 512], F32, tag="p3")

                    def emit_score(i):
                        u = i - 4 * j
                        fs = 0 if u < 0 else 128 * u
                        ps_s = P.tile(
                            [128, 512], F32, tag=f"p{SROT[i % 4]}",
                            name=f"pss{SROT[i % 4]}",
                        )
                        qslice = qT[h][:, j * 512 + fs:(j + 1) * 512]
                        if u >= 0:
                            # diagonal: add -BIG upper-triangle via PE so exp -> 0
                            nc.tensor.matmul(
                                ps_s[:, fs:512], kT[:, i * 128:(i + 1) * 128],
                                qslice, start=True, stop=False,
                            )
                            nc.tensor.matmul(
                                ps_s[:, fs:fs + 128], mnegb[:], idz[:, 0:128],
                                start=False, stop=True,
                            )
                        else:
                            nc.tensor.matmul(
                                ps_s[:, fs:512], kT[:, i * 128:(i + 1) * 128], qslice
                            )
                        et = etp.tile([128, 512], BF16, tag="et")
                        nc.scalar.activation(
                            et[:, fs:512], ps_s[:, fs:512],
                            mybir.ActivationFunctionType.Exp, scale=float(SCALE),
                        )
                        return et, fs

                    pend = {}
                    for i in range(min(LOOK, nblk)):
                        pend[i] = emit_score(i)
                    for i in range(nblk):
                        if i + LOOK < nblk:
                            pend[i + LOOK] = emit_score(i + LOOK)
                        et, fs = pend.pop(i)
                        st = dict(start=(i == 0), stop=(i == nblk - 1))
                        nc.tensor.matmul(
                            ps_y[:, fs:512], vN[:, i * 128:(i + 1) * 128],
                            et[:, fs:512], **st,
                        )
                        nc.tensor.matmul(
                            ps_den[0:1, fs:512], ones_colb[:], et[:, fs:512], **st
                        )
                    rd = work.tile([1, 512], BF16, tag="rd")
                    with nc.allow_low_precision(reason="feeds PE broadcast"):
                        nc.vector.reciprocal(rd[:], ps_den[0:1, :])
                    ps_rb = P.tile([128, 512], F32, tag="p4")
                    nc.tensor.matmul(ps_rb[:], ones_rowb[:], rd[:])
                    yb = work.tile([128, 512], BF16, tag="yb")
                    nc.scalar.activation(
                        yb[:], ps_y[:], mybir.ActivationFunctionType.Copy, scale=1.0
                    )
                    nc.vector.tensor_mul(yTj[h][:], yb[:], ps_rb[:])

                # ===== o_proj partial for chunk j + ReduceScatter =====
                if "F" not in phases:
                    continue
                for mp in range(NM // 2):
                    ob2 = work.tile([128, 1024], BF16, tag=f"ob{mp % 3}")
                    for half in range(2):
                        m = 2 * mp + half
                        ps_o = P.tile([128, 512], F32, tag=f"p{5 + m % 3}")
                        for h in range(HPC):
                            nc.tensor.matmul(
                                ps_o[:],
                                wo_sb[:, (m * HPC + h) * 128:(m * HPC + h + 1) * 128],
                                yTj[h][:], start=(h == 0), stop=(h == HPC - 1),
                            )
                        dst = ob2[:, half * 512:(half + 1) * 512]
                        nc.vector.tensor_copy(dst, ps_o[:])
                    nc.sync.dma_start(
                        part[j][2 * mp * 128:(2 * mp + 2) * 128, :].rearrange(
                            "(two r) c -> r two c", two=2
                        ),
                        ob2[:],
                    )
                if collectives:
                    nc.gpsimd.collective_compute(
                        "ReduceScatter", mybir.AluOpType.add, replica_groups=rg,
                        ins=[part[j][:].opt()], outs=[rsb[j][:].opt()],
                    )

        if bench_reps:
            with tc.For_i(0, bench_reps, 1):
                body()
        else:
            body()
            if collectives and "F" in phases:
                for j in range(NT):
                    nc.sync.dma_start(rs_out[j][:, :], rsb[j][:])

    split_multiwaits(nc)
    return nc


# ---------------------------------------------------------------------------
# host side
# ---------------------------------------------------------------------------

_RUNNER_CACHE = None


def _make_runner(nc, n_cores=NCORES):
    """Build the sharded jit once; returns run(in_maps) -> list of out dicts."""
    import jax
    from jax.sharding import Mesh, NamedSharding, PartitionSpec
    from jax.experimental.shard_map import shard_map
    from concourse import bass2jax
    from concourse.bass2jax import _bass_exec_p, partition_id_tensor

    bass2jax.install_neuronx_cc_hook()

    partition_name = nc.partition_id_tensor.name if nc.partition_id_tensor else None
    in_names, out_names, out_avals, zero_outs = [], [], [], []
    for alloc in nc.m.functions[0].allocations:
        if not isinstance(alloc, mybir.MemoryLocationSet):
            continue
        name = alloc.memorylocations[0].name
        if alloc.kind == "ExternalInput":
            if name != partition_name:
                in_names.append(name)
        elif alloc.kind == "ExternalOutput":
            out_names.append(name)
            shape = tuple(alloc.tensor_shape)
            dtype = mybir.dt.np(alloc.dtype)
            out_avals.append(jax.core.ShapedArray(shape, dtype))
            zero_outs.append(np.zeros(shape, dtype))
    n_params = len(in_names)
    n_outs = len(out_avals)
    all_in_names = list(in_names) + list(out_names)
    if partition_name is not None:
        all_in_names.append(partition_name)
    donate = tuple(range(n_params, n_params + n_outs))

    def _body(*args):
        operands = list(args)
        if partition_name is not None:
            operands.append(partition_id_tensor())
        outs = _bass_exec_p.bind(
            *operands,
            out_avals=tuple(out_avals),
            in_names=tuple(all_in_names),
            out_names=tuple(out_names),
            lowering_input_output_aliases=(),
            sim_require_finite=True,
            sim_require_nnan=True,
            nc=nc,
        )
        return tuple(outs)

    devices = jax.devices()[:n_cores]
    mesh = Mesh(np.asarray(devices), ("core",))
    sharded = jax.jit(
        shard_map(
            _body, mesh=mesh,
            in_specs=(PartitionSpec("core"),) * (n_params + n_outs),
            out_specs=(PartitionSpec("core"),) * n_outs,
            check_rep=False,
        ),
        donate_argnums=donate,
        keep_unused=True,
    )
    shard = NamedSharding(mesh, PartitionSpec("core"))
    zshapes = [((n_cores * z.shape[0],) + z.shape[1:], z.dtype) for z in zero_outs]

    def run(in_maps):
        concat_in = [
            jax.device_put(
                np.concatenate(
                    [np.asarray(in_maps[c][n]) for c in range(n_cores)], axis=0
                ),
                shard,
            )
            for n in in_names
        ]
        zs = [jax.device_put(np.zeros(s, d), shard) for s, d in zshapes]
        outs = sharded(*concat_in, *zs)
        return [
            {
                name: np.asarray(outs[i]).reshape(n_cores, *out_avals[i].shape)[c]
                for i, name in enumerate(out_names)
            }
            for c in range(n_cores)
        ]

    return run


def _get_runner():
    global _RUNNER_CACHE
    if _RUNNER_CACHE is None:
        _RUNNER_CACHE = _make_runner(build_program())
    return _RUNNER_CACHE


def make_inputs(x, input_pos, Wq, Wk, Wv, Wo, q_norm_w, k_norm_w):
    """Host-side sharding / layout prep. Returns per-core input maps."""
    bf16 = _bf16()
    x2d = np.ascontiguousarray(np.asarray(x, np.float32).reshape(T, C)).astype(bf16)
    Wq = np.asarray(Wq, np.float32)
    Wk = np.asarray(Wk, np.float32)
    Wv = np.asarray(Wv, np.float32)
    Wo = np.asarray(Wo, np.float32)
    q_norm_w = np.asarray(q_norm_w, np.float32)
    k_norm_w = np.asarray(k_norm_w, np.float32)
    pos = np.asarray(input_pos, np.float32)

    # interleaved head-dim permutation: [0, 64, 1, 65, ...]
    perm = np.empty(128, np.int64)
    perm[0::2] = np.arange(64)
    perm[1::2] = np.arange(64) + 64
    pswap = np.arange(128) ^ 1  # adjacent-pair swap of interleaved rows

    # rope tables in interleaved layout (sign of the rotate-half folded in),
    # with the rmsnorm weight folded in: the even/odd rows of the sin table
    # carry the weight of the PAIRED row (the shuffled operand).
    inv_freq = (THETA ** (-(np.arange(0, D, 2, dtype=np.float32)) / D)).astype(
        np.float32
    )
    fr = pos[:, None] * inv_freq[None, :]  # [T, 64]
    cos = np.cos(fr).astype(np.float32).T  # [64, T]
    sin = np.sin(fr).astype(np.float32).T
    cos_il = np.empty((128, T), np.float32)
    cos_il[0::2] = cos
    cos_il[1::2] = cos
    sin_eff = np.empty((128, T), np.float32)
    sin_eff[0::2] = -sin
    sin_eff[1::2] = sin

    def fold(w):
        wp = w[perm]
        wc = np.ascontiguousarray(cos_il * wp[:, None]).astype(bf16)
        ws = np.ascontiguousarray(sin_eff * wp[pswap][:, None]).astype(bf16)
        return wc, ws

    wqc_h, wqs_h = fold(q_norm_w)
    wkc_h, wks_h = fold(k_norm_w)

    ident_h = np.eye(128, dtype=np.float32).astype(bf16)
    # mneg[r, c] = -BIG iff r < c; with rhs=[I|0] this adds -BIG to score[p, x]
    # for x < p (future positions) so exp underflows to exactly 0.
    rr, cc2 = np.meshgrid(np.arange(128), np.arange(128), indexing="ij")
    mneg_h = np.where(rr < cc2, np.float32(-1e30), np.float32(0)).astype(bf16)

    Wq4 = Wq.reshape(N_HEAD, D, C)
    Wk4 = Wk.reshape(N_KV, D, C)
    Wv4 = Wv.reshape(N_KV, D, C)

    in_maps = []
    for c in range(NCORES):
        g = c // 2
        Wc = Wq4[HPC * c:HPC * (c + 1)][:, perm, :]  # [4, 128, C]
        wq_host = np.ascontiguousarray(
            Wc.reshape(HPC, 128, NK, 128).transpose(3, 0, 2, 1).reshape(128, -1)
        ).astype(bf16)
        wk_host = np.ascontiguousarray(
            Wk4[g][perm].reshape(128, NK, 128).transpose(2, 1, 0).reshape(128, -1)
        ).astype(bf16)
        wv_host = np.ascontiguousarray(
            Wv4[g].reshape(128, NK, 128).transpose(2, 1, 0).reshape(128, -1)
        ).astype(bf16)
        # o_proj lhsT per (m-tile, local head): wo_host[d, (m*4+h)*128 + i]
        # = Wo[128m+i, 512c + 128h + d]
        WoC = Wo[:, 512 * c:512 * (c + 1)]  # [2048, 512]
        wo_host = np.ascontiguousarray(
            WoC.reshape(NM, 128, HPC, 128).transpose(3, 0, 2, 1).reshape(128, -1)
        ).astype(bf16)
        in_maps.append(
            {
                "xb": x2d,
                "wq": wq_host,
                "wk": wk_host,
                "wv": wv_host,
                "wo": wo_host,
                "wqc": wqc_h,
                "wqs": wqs_h,
                "wkc": wkc_h,
                "wks": wks_h,
                "identp": ident_h,
                "mnegp": mneg_h,
            }
        )
    return in_maps


def kernel(x, input_pos, Wq, Wk, Wv, Wo, q_norm_w, k_norm_w):
    run = _get_runner()
    in_maps = make_inputs(x, input_pos, Wq, Wk, Wv, Wo, q_norm_w, k_norm_w)
    results = run(in_maps)
    out = np.empty((1, T, C), np.float32)
    for c in range(NCORES):
        for j in range(NT):
            out[0][j * 512:(j + 1) * 512, 256 * c:256 * (c + 1)] = (
                results[c][f"rs{j}"].astype(np.float32).T
            )
    return out


# revision 47
# speedup vs baseline: 2.6975x; 1.0291x over previous
"""Trainium2 Bass kernel for causal self-attention (GQA, RoPE, q/k-RMSNorm).

Sharding: tensor-parallel over heads across 8 cores.
  - core c owns q-heads [4c, 4c+4) and kv-head c//2 (each kv head serves 8 q heads)
  - x is pre-transposed on the host; each core keeps the full x^T in SBUF,
    packed as 4 tiles of 4 k-chunks so one DMA loads 4 chunks per T-slice
  - attention is computed transposed (E^T = exp(K.Q^T)) so V in natural [S,D]
    layout (built via XBAR DMA-transpose) is the matmul lhsT and y^T comes
    out in [D,T] layout directly
  - o_proj is computed LOCALLY per core: partial O^T = sum_h Wo[:,head h] @ y^T_h
    over the core's 4 heads, then ONE ReduceScatter per 512-column T-chunk
    sums partials across cores and leaves core c with O^T rows [256c,256c+256).
    The 4 chunked ReduceScatters overlap with the next chunk's attention; only
    the last one is exposed at the tail.
  - the whole loop is j-chunked (T in 4 chunks of 512): QKV+norm+rope ->
    attention (4 heads) -> o_proj partial -> ReduceScatter, all per chunk
  - the causal mask is applied ON the PE: diagonal blocks accumulate a -1e30
    upper-triangle (lhsT=mneg, rhs=[I|0]) into the score PSUM so exp -> 0,
    keeping the exp -> y chain free of extra DVE work; diagonal blocks also
    shrink their free dim to skip fully-masked columns
  - attention runs with a 3-block score lookahead over 4 rotating PSUM banks
    so PE always has score work to issue while Act computes exps
  - head-dim rows of q/k are interleaved (d -> [0,64,1,65,...]) so the RoPE
    rotate-half becomes an adjacent-pair partition swap (one stream_shuffle);
    the rmsnorm weight is folded into the host-precomputed cos/sin tables;
    the 1/rms and 1/den broadcasts over partitions use rank-1 PE matmuls
  - everything flows in bf16 (f32 PSUM accumulation), incl. the output

Engines: PE does all matmuls + transposed-mask adds; Act does PSUM->SBUF
copies + sqrt/exp; DVE does the rope chain, reciprocals, o_proj PSUM->SBUF
copies and final scaling; collectives are dispatched on gpsimd; DMAs are
batched (paired o_proj writes, 4-chunk x^T loads) to amortize the ~0.6us
per-instruction HWDGE/SEQ dispatch overhead.
"""

import sys

sys.path.insert(0, "/opt/trn_rl_repo")

from contextlib import ExitStack

import numpy as np

import bass_rust
import concourse.bass as bass
import concourse.mybir as mybir
from concourse import tile

F32 = mybir.dt.float32
BF16 = mybir.dt.bfloat16

N_HEAD = 32
N_KV = 4
D = 128
C = 2048
T = 2048
NCORES = 8
HPC = N_HEAD // NCORES  # q heads per core = 4
THETA = 1000000.0
EPS = 1e-6
SCALE = 1.0 / np.sqrt(128.0)

NT = T // 512  # 4 T-chunks of 512
NK = C // 128  # 16 contraction tiles for qkv
NS = T // 128  # 16 S-blocks of 128
NM = C // 128  # 16 output m-tiles for o_proj

# stream_shuffle swaps within each 32-partition quadrant; adjacent-pair swap
SWAP_MASK = [i ^ 1 for i in range(32)]

_BF16_NP = None


def _bf16():
    global _BF16_NP
    if _BF16_NP is None:
        import ml_dtypes

        _BF16_NP = np.dtype(ml_dtypes.bfloat16)
    return _BF16_NP


def split_multiwaits(nc):
    """The walrus build in this container supports one sync-wait per
    instruction; hoist extra waits onto NOPs inserted before the offender."""
    ctr = 0
    for f in nc.m.functions:
        for bb in f.blocks:
            new_insts = []
            changed = False
            for inst in bb.instructions:
                si = inst.sync_info
                if si is not None and si.on_wait and len(si.on_wait) > 1:
                    waits = list(si.on_wait)
                    for w in waits[:-1]:
                        ctr += 1
                        nop = bass_rust.InstNoOp(name=f"splitw-{ctr}", ins=[], outs=[])
                        nop.engine = inst.engine
                        nop.sync_info = bass_rust.SyncInfo(on_wait=[w], on_update=[])
                        new_insts.append(nop)
                    inst.sync_info = bass_rust.SyncInfo(
                        on_wait=[waits[-1]], on_update=list(si.on_update or [])
                    )
                    changed = True
                new_insts.append(inst)
            if changed:
                bb.instructions = new_insts


def build_program(bench_reps=0, phases="ABDF"):
    nc = bass.Bass("TRN2", target_bir_lowering=False, debug=False, num_devices=NCORES)

    xb = nc.declare_dram_parameter("xb", [C, T], BF16, isOutput=False)
    wq = nc.declare_dram_parameter("wq", [128, HPC * NK * 128], BF16, isOutput=False)
    wk = nc.declare_dram_parameter("wk", [128, NK * 128], BF16, isOutput=False)
    wv = nc.declare_dram_parameter("wv", [128, NK * 128], BF16, isOutput=False)
    wo = nc.declare_dram_parameter("wo", [128, NM * HPC * 128], BF16, isOutput=False)
    wqc = nc.declare_dram_parameter("wqc", [128, T], BF16, isOutput=False)
    wqs = nc.declare_dram_parameter("wqs", [128, T], BF16, isOutput=False)
    wkc = nc.declare_dram_parameter("wkc", [128, T], BF16, isOutput=False)
    wks = nc.declare_dram_parameter("wks", [128, T], BF16, isOutput=False)
    identp = nc.declare_dram_parameter("identp", [128, 128], BF16, isOutput=False)
    mnegp = nc.declare_dram_parameter("mnegp", [128, 128], BF16, isOutput=False)
    rs_out = [
        nc.declare_dram_parameter(f"rs{j}", [2 * 128, 512], BF16, isOutput=True)
        for j in range(NT)
    ]

    rg = [list(range(NCORES))]
    collectives = bench_reps == 0

    with tile.TileContext(nc) as tc, ExitStack() as ctx:
        const = ctx.enter_context(tc.tile_pool(name="const", bufs=1))
        wpool = ctx.enter_context(tc.tile_pool(name="wpool", bufs=1))
        act = ctx.enter_context(tc.tile_pool(name="act", bufs=1))
        work = ctx.enter_context(tc.tile_pool(name="work", bufs=2))
        etp = ctx.enter_context(tc.tile_pool(name="etp", bufs=5))
        P = ctx.enter_context(tc.tile_pool(name="P", bufs=1, space="PSUM"))
        dram = ctx.enter_context(tc.tile_pool(name="dram", bufs=1, space="DRAM"))

        # ---- constants ----
        ones_colb = const.tile([128, 1], BF16)
        nc.vector.memset(ones_colb[:], 1.0)
        ones_rowb = const.tile([1, 128], BF16)
        nc.vector.memset(ones_rowb[:], 1.0)
        eps_col = const.tile([128, 1], F32)
        nc.vector.memset(eps_col[:], EPS)
        mnegb = const.tile([128, 128], BF16)
        # [I | 0]: rhs for the PE causal-mask add (cols >= 128 contribute 0)
        idz = const.tile([128, 512], BF16)
        nc.vector.memset(idz[:], 0.0)

        # ---- resident weights / tables / activations ----
        wq_sb = wpool.tile([128, HPC * NK * 128], BF16)
        wk_sb = wpool.tile([128, NK * 128], BF16)
        wv_sb = wpool.tile([128, NK * 128], BF16)
        wo_sb = wpool.tile([128, NM * HPC * 128], BF16)
        tqc = wpool.tile([128, T], BF16)
        tqs = wpool.tile([128, T], BF16)
        tkc = wpool.tile([128, T], BF16)
        tks = wpool.tile([128, T], BF16)

        xT = [act.tile([128, T], BF16, name=f"xT{k}") for k in range(NK)]
        qT = [act.tile([128, T], BF16, name=f"qT{h}") for h in range(HPC)]
        kT = act.tile([128, T], BF16)
        vN = act.tile([128, NS * 128], BF16)  # natural [S,D] as 16 s-tiles
        yTj = [act.tile([128, 512], BF16, name=f"yTj{h}") for h in range(HPC)]

        part = [dram.tile([C, 512], BF16, name=f"part{j}") for j in range(NT)]
        rsb = [dram.tile([2 * 128, 512], BF16, name=f"rsb{j}") for j in range(NT)]

        # upfront loads, ordered by first use (nc.sync queue, in-order).
        # xT for chunks j>0 is prefetched inside the j-loop (one chunk ahead)
        # so the per-chunk vN transposes don't queue behind them.
        nc.sync.dma_start(wv_sb[:], wv[:, :])
        nc.sync.dma_start(wk_sb[:], wk[:, :])
        nc.sync.dma_start(wq_sb[:], wq[:, :])
        if "A" in phases:
            for k in range(NK):
                nc.sync.dma_start(
                    xT[k][:, 0:512], xb[k * 128:(k + 1) * 128, 0:512]
                )
        nc.sync.dma_start(tkc[:], wkc[:, :])
        nc.sync.dma_start(tks[:], wks[:, :])
        nc.sync.dma_start(tqc[:], wqc[:, :])
        nc.sync.dma_start(tqs[:], wqs[:, :])
        nc.sync.dma_start(wo_sb[:], wo[:, :])
        nc.sync.dma_start(mnegb[:], mnegp[:, :])
        nc.sync.dma_start(idz[:, 0:128], identp[:, :])

        def norm_rope(ps, wcos, wsin, dest, js):
            """dest[:, js] = rope(rmsnorm-scaled ps); weight folded in tables."""
            raw = work.tile([128, 512], BF16, tag="raw")
            nc.scalar.activation(
                raw[:], ps[:], mybir.ActivationFunctionType.Copy, scale=1.0
            )
            sqr = work.tile([128, 512], BF16, tag="sqr")
            nc.vector.tensor_mul(sqr[:], raw[:], raw[:])
            ssq = P.tile([128, 512], F32, tag="p6")
            nc.tensor.matmul(ssq[0:1, :], ones_colb[:], sqr[:])
            rms = work.tile([1, 512], F32, tag="rms")
            nc.scalar.activation(
                rms[:], ssq[0:1, :], mybir.ActivationFunctionType.Sqrt,
                scale=1.0 / 128.0, bias=eps_col[0:1, :],
            )
            rinv = work.tile([1, 512], BF16, tag="rinv")
            with nc.allow_low_precision(reason="feeds PE broadcast"):
                nc.vector.reciprocal(rinv[:], rms[:])
            rb = P.tile([128, 512], F32, tag="p7")
            nc.tensor.matmul(rb[:], ones_rowb[:], rinv[:])
            qs = work.tile([128, 512], BF16, tag="qs")
            nc.vector.stream_shuffle(qs[:], raw[:], mask=SWAP_MASK)
            t1 = work.tile([128, 512], BF16, tag="t1")
            nc.vector.tensor_mul(t1[:], raw[:], wcos[:, js])
            t2 = work.tile([128, 512], BF16, tag="t2")
            nc.vector.tensor_mul(t2[:], qs[:], wsin[:, js])
            pre = work.tile([128, 512], BF16, tag="pre")
            nc.vector.tensor_add(pre[:], t1[:], t2[:])
            nc.vector.tensor_mul(dest[:, js], pre[:], rb[:])

        def body():
            for j in range(NT):
                js = slice(j * 512, (j + 1) * 512)

                # ===== QKV projection + RMSNorm + RoPE for chunk j =====
                # K and V first so kT/vN are ready when attention starts.
                if "B" in phases:
                    ps_v = P.tile([128, 512], F32, tag="p5")
                    for k in range(NK):
                        nc.tensor.matmul(
                            ps_v[:], wv_sb[:, k * 128:(k + 1) * 128], xT[k][:, js],
                            start=(k == 0), stop=(k == NK - 1),
                        )
                    # v: [D, T]-chunk -> natural [S, D] tiles via XBAR transpose
                    vt = work.tile([128, 512], BF16, tag="vt")
                    nc.scalar.activation(
                        vt[:], ps_v[:], mybir.ActivationFunctionType.Copy, scale=1.0
                    )
                    for u in range(4):
                        s_tile = j * 4 + u
                        nc.sync.dma_start(
                            vN[:, s_tile * 128:(s_tile + 1) * 128],
                            vt[:, u * 128:(u + 1) * 128], transpose=True,
                        )
                    if "A" in phases and j == 0:
                        for k in range(NK):
                            nc.sync.dma_start(
                                xT[k][:, 512:T], xb[k * 128:(k + 1) * 128, 512:T]
                            )
                    ps_k = P.tile([128, 512], F32, tag="p4")
                    for k in range(NK):
                        nc.tensor.matmul(
                            ps_k[:], wk_sb[:, k * 128:(k + 1) * 128], xT[k][:, js],
                            start=(k == 0), stop=(k == NK - 1),
                        )
                    norm_rope(ps_k, tkc, tks, kT, js)
                    ps_q = [
                        P.tile([128, 512], F32, tag=f"p{h}", name=f"psq{h}")
                        for h in range(HPC)
                    ]
                    for h in range(HPC):
                        for k in range(NK):
                            nc.tensor.matmul(
                                ps_q[h][:],
                                wq_sb[:, (h * NK + k) * 128:(h * NK + k + 1) * 128],
                                xT[k][:, js], start=(k == 0), stop=(k == NK - 1),
                            )
                        norm_rope(ps_q[h], tqc, tqs, qT[h], js)

                # ===== attention for chunk j, all 4 local heads =====
                if "D" not in phases:
                    continue
                nblk = 4 * j + 4
                LOOK = 3
                SROT = (0, 1, 5, 6)  # rotating PSUM banks for score lookahead

                def emit_score(h, i):
                    u = i - 4 * j
                    fs = 0 if u < 0 else 128 * u
                    ps_s = P.tile(
                        [128, 512], F32, tag=f"p{SROT[i % 4]}",
                        name=f"pss{SROT[i % 4]}",
                    )
                    qslice = qT[h][:, j * 512 + fs:(j + 1) * 512]
                    if u >= 0:
                        # diagonal: add -BIG upper-triangle via PE so exp -> 0
                        nc.tensor.matmul(
                            ps_s[:, fs:512], kT[:, i * 128:(i + 1) * 128],
                            qslice, start=True, stop=False,
                        )
                        nc.tensor.matmul(
                            ps_s[:, fs:fs + 128], mnegb[:], idz[:, 0:128],
                            start=False, stop=True,
                        )
                    else:
                        nc.tensor.matmul(
                            ps_s[:, fs:512], kT[:, i * 128:(i + 1) * 128], qslice
                        )
                    et = etp.tile([128, 512], BF16, tag="et")
                    nc.scalar.activation(
                        et[:, fs:512], ps_s[:, fs:512],
                        mybir.ActivationFunctionType.Exp, scale=float(SCALE),
                    )
                    return et, fs

                def make_tail(ps_y, ps_den, h):
                    def tail():
                        rd = work.tile([1, 512], BF16, tag="rd")
                        with nc.allow_low_precision(reason="feeds PE broadcast"):
                            nc.vector.reciprocal(rd[:], ps_den[0:1, :])
                        ps_rb = P.tile([128, 512], F32, tag="p4")
                        nc.tensor.matmul(ps_rb[:], ones_rowb[:], rd[:])
                        yb = work.tile([128, 512], BF16, tag="yb")
                        nc.scalar.activation(
                            yb[:], ps_y[:],
                            mybir.ActivationFunctionType.Copy, scale=1.0,
                        )
                        nc.vector.tensor_mul(yTj[h][:], yb[:], ps_rb[:])
                    return tail

                for h in range(HPC):
                    ps_y = P.tile([128, 512], F32, tag="p2")
                    ps_den = P.tile([128, 512], F32, tag="p3")
                    pend = {}
                    for i in range(min(LOOK, nblk)):
                        pend[i] = emit_score(h, i)
                    for i in range(nblk):
                        if i + LOOK < nblk:
                            pend[i + LOOK] = emit_score(h, i + LOOK)
                        et, fs = pend.pop(i)
                        st = dict(start=(i == 0), stop=(i == nblk - 1))
                        nc.tensor.matmul(
                            ps_y[:, fs:512], vN[:, i * 128:(i + 1) * 128],
                            et[:, fs:512], **st,
                        )
                        nc.tensor.matmul(
                            ps_den[0:1, fs:512], ones_colb[:], et[:, fs:512], **st
                        )
                    make_tail(ps_y, ps_den, h)()

                # ===== o_proj partial for chunk j + ReduceScatter =====
                if "F" not in phases:
                    continue
                for mp in range(NM // 2):
                    ob2 = work.tile([128, 1024], BF16, tag=f"ob{mp % 3}")
                    for half in range(2):
                        m = 2 * mp + half
                        ps_o = P.tile([128, 512], F32, tag=f"p{5 + m % 3}")
                        for h in range(HPC):
                            nc.tensor.matmul(
                                ps_o[:],
                                wo_sb[:, (m * HPC + h) * 128:(m * HPC + h + 1) * 128],
                                yTj[h][:], start=(h == 0), stop=(h == HPC - 1),
                            )
                        dst = ob2[:, half * 512:(half + 1) * 512]
                        nc.vector.tensor_copy(dst, ps_o[:])
                    nc.sync.dma_start(
                        part[j][2 * mp * 128:(2 * mp + 2) * 128, :].rearrange(
                            "(two r) c -> r two c", two=2
                        ),
                        ob2[:],
                    )
                if collectives:
                    nc.gpsimd.collective_compute(
                        "ReduceScatter", mybir.AluOpType.add, replica_groups=rg,
                        ins=[part[j][:].opt()], outs=[rsb[j][:].opt()],
                    )

        if bench_reps:
            with tc.For_i(0, bench_reps, 1):
                body()
        else:
            body()
            if collectives and "F" in phases:
                for j in range(NT):
                    nc.sync.dma_start(rs_out[j][:, :], rsb[j][:])

    split_multiwaits(nc)
    return nc


# ---------------------------------------------------------------------------
# host side
# ---------------------------------------------------------------------------

_RUNNER_CACHE = None


def _make_runner(nc, n_cores=NCORES):
    """Build the sharded jit once; returns run(in_maps) -> list of out dicts."""
    import jax
    from jax.sharding import Mesh, NamedSharding, PartitionSpec
    from jax.experimental.shard_map import shard_map
    from concourse import bass2jax
    from concourse.bass2jax import _bass_exec_p, partition_id_tensor

    bass2jax.install_neuronx_cc_hook()

    partition_name = nc.partition_id_tensor.name if nc.partition_id_tensor else None
    in_names, out_names, out_avals, zero_outs = [], [], [], []
    for alloc in nc.m.functions[0].allocations:
        if not isinstance(alloc, mybir.MemoryLocationSet):
            continue
        name = alloc.memorylocations[0].name
        if alloc.kind == "ExternalInput":
            if name != partition_name:
                in_names.append(name)
        elif alloc.kind == "ExternalOutput":
            out_names.append(name)
            shape = tuple(alloc.tensor_shape)
            dtype = mybir.dt.np(alloc.dtype)
            out_avals.append(jax.core.ShapedArray(shape, dtype))
            zero_outs.append(np.zeros(shape, dtype))
    n_params = len(in_names)
    n_outs = len(out_avals)
    all_in_names = list(in_names) + list(out_names)
    if partition_name is not None:
        all_in_names.append(partition_name)
    donate = tuple(range(n_params, n_params + n_outs))

    def _body(*args):
        operands = list(args)
        if partition_name is not None:
            operands.append(partition_id_tensor())
        outs = _bass_exec_p.bind(
            *operands,
            out_avals=tuple(out_avals),
            in_names=tuple(all_in_names),
            out_names=tuple(out_names),
            lowering_input_output_aliases=(),
            sim_require_finite=True,
            sim_require_nnan=True,
            nc=nc,
        )
        return tuple(outs)

    devices = jax.devices()[:n_cores]
    mesh = Mesh(np.asarray(devices), ("core",))
    sharded = jax.jit(
        shard_map(
            _body, mesh=mesh,
            in_specs=(PartitionSpec("core"),) * (n_params + n_outs),
            out_specs=(PartitionSpec("core"),) * n_outs,
            check_rep=False,
        ),
        donate_argnums=donate,
        keep_unused=True,
    )
    shard = NamedSharding(mesh, PartitionSpec("core"))
    zshapes = [((n_cores * z.shape[0],) + z.shape[1:], z.dtype) for z in zero_outs]

    def run(in_maps):
        concat_in = [
            jax.device_put(
                np.concatenate(
                    [np.asarray(in_maps[c][n]) for c in range(n_cores)], axis=0
                ),
                shard,
            )
            for n in in_names
        ]
        zs = [jax.device_put(np.zeros(s, d), shard) for s, d in zshapes]
        outs = sharded(*concat_in, *zs)
        return [
            {
                name: np.asarray(outs[i]).reshape(n_cores, *out_avals[i].shape)[c]
                for i, name in enumerate(out_names)
            }
            for c in range(n_cores)
        ]

    return run


def _get_runner():
    global _RUNNER_CACHE
    if _RUNNER_CACHE is None:
        _RUNNER_CACHE = _make_runner(build_program())
    return _RUNNER_CACHE


def make_inputs(x, input_pos, Wq, Wk, Wv, Wo, q_norm_w, k_norm_w):
    """Host-side sharding / layout prep. Returns per-core input maps."""
    bf16 = _bf16()
    x2d = np.ascontiguousarray(
        np.asarray(x, np.float32).reshape(T, C).T
    ).astype(bf16)  # pre-transposed: [C, T]
    Wq = np.asarray(Wq, np.float32)
    Wk = np.asarray(Wk, np.float32)
    Wv = np.asarray(Wv, np.float32)
    Wo = np.asarray(Wo, np.float32)
    q_norm_w = np.asarray(q_norm_w, np.float32)
    k_norm_w = np.asarray(k_norm_w, np.float32)
    pos = np.asarray(input_pos, np.float32)

    # interleaved head-dim permutation: [0, 64, 1, 65, ...]
    perm = np.empty(128, np.int64)
    perm[0::2] = np.arange(64)
    perm[1::2] = np.arange(64) + 64
    pswap = np.arange(128) ^ 1  # adjacent-pair swap of interleaved rows

    # rope tables in interleaved layout (sign of the rotate-half folded in),
    # with the rmsnorm weight folded in: the even/odd rows of the sin table
    # carry the weight of the PAIRED row (the shuffled operand).
    inv_freq = (THETA ** (-(np.arange(0, D, 2, dtype=np.float32)) / D)).astype(
        np.float32
    )
    fr = pos[:, None] * inv_freq[None, :]  # [T, 64]
    cos = np.cos(fr).astype(np.float32).T  # [64, T]
    sin = np.sin(fr).astype(np.float32).T
    cos_il = np.empty((128, T), np.float32)
    cos_il[0::2] = cos
    cos_il[1::2] = cos
    sin_eff = np.empty((128, T), np.float32)
    sin_eff[0::2] = -sin
    sin_eff[1::2] = sin

    def fold(w):
        wp = w[perm]
        wc = np.ascontiguousarray(cos_il * wp[:, None]).astype(bf16)
        ws = np.ascontiguousarray(sin_eff * wp[pswap][:, None]).astype(bf16)
        return wc, ws

    wqc_h, wqs_h = fold(q_norm_w)
    wkc_h, wks_h = fold(k_norm_w)

    ident_h = np.eye(128, dtype=np.float32).astype(bf16)
    # mneg[r, c] = -BIG iff r < c; with rhs=[I|0] this adds -BIG to score[p, x]
    # for x < p (future positions) so exp underflows to exactly 0.
    rr, cc2 = np.meshgrid(np.arange(128), np.arange(128), indexing="ij")
    mneg_h = np.where(rr < cc2, np.float32(-1e30), np.float32(0)).astype(bf16)

    Wq4 = Wq.reshape(N_HEAD, D, C)
    Wk4 = Wk.reshape(N_KV, D, C)
    Wv4 = Wv.reshape(N_KV, D, C)

    in_maps = []
    for c in range(NCORES):
        g = c // 2
        Wc = Wq4[HPC * c:HPC * (c + 1)][:, perm, :]  # [4, 128, C]
        wq_host = np.ascontiguousarray(
            Wc.reshape(HPC, 128, NK, 128).transpose(3, 0, 2, 1).reshape(128, -1)
        ).astype(bf16)
        wk_host = np.ascontiguousarray(
            Wk4[g][perm].reshape(128, NK, 128).transpose(2, 1, 0).reshape(128, -1)
        ).astype(bf16)
        wv_host = np.ascontiguousarray(
            Wv4[g].reshape(128, NK, 128).transpose(2, 1, 0).reshape(128, -1)
        ).astype(bf16)
        # o_proj lhsT per (m-tile, local head): wo_host[d, (m*4+h)*128 + i]
        # = Wo[128m+i, 512c + 128h + d]
        WoC = Wo[:, 512 * c:512 * (c + 1)]  # [2048, 512]
        wo_host = np.ascontiguousarray(
            WoC.reshape(NM, 128, HPC, 128).transpose(3, 0, 2, 1).reshape(128, -1)
        ).astype(bf16)
        in_maps.append(
            {
                "xb": x2d,
                "wq": wq_host,
                "wk": wk_host,
                "wv": wv_host,
                "wo": wo_host,
                "wqc": wqc_h,
                "wqs": wqs_h,
                "wkc": wkc_h,
                "wks": wks_h,
                "identp": ident_h,
                "mnegp": mneg_h,
            }
        )
    return in_maps


def kernel(x, input_pos, Wq, Wk, Wv, Wo, q_norm_w, k_norm_w):
    run = _get_runner()
    in_maps = make_inputs(x, input_pos, Wq, Wk, Wv, Wo, q_norm_w, k_norm_w)
    results = run(in_maps)
    out = np.empty((1, T, C), np.float32)
    for c in range(NCORES):
        for j in range(NT):
            out[0][j * 512:(j + 1) * 512, 256 * c:256 * (c + 1)] = (
                results[c][f"rs{j}"].astype(np.float32).T
            )
    return out


# revision 50
# speedup vs baseline: 2.7153x; 1.0066x over previous
"""Trainium2 Bass kernel for causal self-attention (GQA, RoPE, q/k-RMSNorm).

Sharding: tensor-parallel over heads across 8 cores.
  - core c owns q-heads [4c, 4c+4) and kv-head c//2 (each kv head serves 8 q heads)
  - x is pre-transposed on the host; each core keeps the full x^T in SBUF,
    packed as 4 tiles of 4 k-chunks so one DMA loads 4 chunks per T-slice
  - attention is computed transposed (E^T = exp(K.Q^T)) so V in natural [S,D]
    layout (built via XBAR DMA-transpose) is the matmul lhsT and y^T comes
    out in [D,T] layout directly
  - o_proj is computed LOCALLY per core: partial O^T = sum_h Wo[:,head h] @ y^T_h
    over the core's 4 heads, then ONE ReduceScatter per 512-column T-chunk
    sums partials across cores and leaves core c with O^T rows [256c,256c+256).
    The 4 chunked ReduceScatters overlap with the next chunk's attention; only
    the last one is exposed at the tail.
  - the whole loop is j-chunked (T in 4 chunks of 512): QKV+norm+rope ->
    attention (4 heads) -> o_proj partial -> ReduceScatter, all per chunk
  - the causal mask is applied ON the PE: diagonal blocks accumulate a -1e30
    upper-triangle (lhsT=mneg, rhs=[I|0]) into the score PSUM so exp -> 0,
    keeping the exp -> y chain free of extra DVE work; diagonal blocks also
    shrink their free dim to skip fully-masked columns
  - attention runs with a 3-block score lookahead over 4 rotating PSUM banks
    so PE always has score work to issue while Act computes exps
  - head-dim rows of q/k are interleaved (d -> [0,64,1,65,...]) so the RoPE
    rotate-half becomes an adjacent-pair partition swap (one stream_shuffle);
    the rmsnorm weight is folded into the host-precomputed cos/sin tables;
    the 1/rms and 1/den broadcasts over partitions use rank-1 PE matmuls
  - everything flows in bf16 (f32 PSUM accumulation), incl. the output

Engines: PE does all matmuls + transposed-mask adds; Act does PSUM->SBUF
copies + sqrt/exp; DVE does the rope chain, reciprocals, o_proj PSUM->SBUF
copies and final scaling; collectives are dispatched on gpsimd; DMAs are
batched (paired o_proj writes, 4-chunk x^T loads) to amortize the ~0.6us
per-instruction HWDGE/SEQ dispatch overhead.
"""

import sys

sys.path.insert(0, "/opt/trn_rl_repo")

from contextlib import ExitStack

import numpy as np

import bass_rust
import concourse.bass as bass
import concourse.mybir as mybir
from concourse import tile

F32 = mybir.dt.float32
BF16 = mybir.dt.bfloat16

N_HEAD = 32
N_KV = 4
D = 128
C = 2048
T = 2048
NCORES = 8
HPC = N_HEAD // NCORES  # q heads per core = 4
THETA = 1000000.0
EPS = 1e-6
SCALE = 1.0 / np.sqrt(128.0)

NT = T // 512  # 4 T-chunks of 512
NK = C // 128  # 16 contraction tiles for qkv
NS = T // 128  # 16 S-blocks of 128
NM = C // 128  # 16 output m-tiles for o_proj

# stream_shuffle swaps within each 32-partition quadrant; adjacent-pair swap
SWAP_MASK = [i ^ 1 for i in range(32)]

_BF16_NP = None


def _bf16():
    global _BF16_NP
    if _BF16_NP is None:
        import ml_dtypes

        _BF16_NP = np.dtype(ml_dtypes.bfloat16)
    return _BF16_NP


def split_multiwaits(nc):
    """The walrus build in this container supports one sync-wait per
    instruction; hoist extra waits onto NOPs inserted before the offender."""
    ctr = 0
    for f in nc.m.functions:
        for bb in f.blocks:
            new_insts = []
            changed = False
            for inst in bb.instructions:
                si = inst.sync_info
                if si is not None and si.on_wait and len(si.on_wait) > 1:
                    waits = list(si.on_wait)
                    for w in waits[:-1]:
                        ctr += 1
                        nop = bass_rust.InstNoOp(name=f"splitw-{ctr}", ins=[], outs=[])
                        nop.engine = inst.engine
                        nop.sync_info = bass_rust.SyncInfo(on_wait=[w], on_update=[])
                        new_insts.append(nop)
                    inst.sync_info = bass_rust.SyncInfo(
                        on_wait=[waits[-1]], on_update=list(si.on_update or [])
                    )
                    changed = True
                new_insts.append(inst)
            if changed:
                bb.instructions = new_insts


def build_program(bench_reps=0, phases="ABDF"):
    nc = bass.Bass("TRN2", target_bir_lowering=False, debug=False, num_devices=NCORES)

    xb = nc.declare_dram_parameter("xb", [C, T], BF16, isOutput=False)
    wq = nc.declare_dram_parameter("wq", [128, HPC * NK * 128], BF16, isOutput=False)
    wk = nc.declare_dram_parameter("wk", [128, NK * 128], BF16, isOutput=False)
    wv = nc.declare_dram_parameter("wv", [128, NK * 128], BF16, isOutput=False)
    wo = nc.declare_dram_parameter("wo", [128, NM * HPC * 128], BF16, isOutput=False)
    wqc = nc.declare_dram_parameter("wqc", [128, T], BF16, isOutput=False)
    wqs = nc.declare_dram_parameter("wqs", [128, T], BF16, isOutput=False)
    wkc = nc.declare_dram_parameter("wkc", [128, T], BF16, isOutput=False)
    wks = nc.declare_dram_parameter("wks", [128, T], BF16, isOutput=False)
    identp = nc.declare_dram_parameter("identp", [128, 128], BF16, isOutput=False)
    mnegp = nc.declare_dram_parameter("mnegp", [128, 128], BF16, isOutput=False)
    rs_out = [
        nc.declare_dram_parameter(f"rs{j}", [2 * 128, 512], BF16, isOutput=True)
        for j in range(NT)
    ]

    rg = [list(range(NCORES))]
    collectives = bench_reps == 0

    with tile.TileContext(nc) as tc, ExitStack() as ctx:
        const = ctx.enter_context(tc.tile_pool(name="const", bufs=1))
        wpool = ctx.enter_context(tc.tile_pool(name="wpool", bufs=1))
        act = ctx.enter_context(tc.tile_pool(name="act", bufs=1))
        work = ctx.enter_context(tc.tile_pool(name="work", bufs=2))
        etp = ctx.enter_context(tc.tile_pool(name="etp", bufs=5))
        P = ctx.enter_context(tc.tile_pool(name="P", bufs=1, space="PSUM"))
        dram = ctx.enter_context(tc.tile_pool(name="dram", bufs=1, space="DRAM"))

        # ---- constants ----
        ones_colb = const.tile([128, 1], BF16)
        nc.vector.memset(ones_colb[:], 1.0)
        ones_rowb = const.tile([1, 128], BF16)
        nc.vector.memset(ones_rowb[:], 1.0)
        eps_col = const.tile([128, 1], F32)
        nc.vector.memset(eps_col[:], EPS)
        mnegb = const.tile([128, 128], BF16)
        # [I | 0]: rhs for the PE causal-mask add (cols >= 128 contribute 0)
        idz = const.tile([128, 512], BF16)
        nc.vector.memset(idz[:], 0.0)

        # ---- resident weights / tables / activations ----
        wq_sb = wpool.tile([128, HPC * NK * 128], BF16)
        wk_sb = wpool.tile([128, NK * 128], BF16)
        wv_sb = wpool.tile([128, NK * 128], BF16)
        wo_sb = wpool.tile([128, NM * HPC * 128], BF16)
        tqc = wpool.tile([128, T], BF16)
        tqs = wpool.tile([128, T], BF16)
        tkc = wpool.tile([128, T], BF16)
        tks = wpool.tile([128, T], BF16)

        xT = [act.tile([128, T], BF16, name=f"xT{k}") for k in range(NK)]
        qT = [act.tile([128, T], BF16, name=f"qT{h}") for h in range(HPC)]
        kT = act.tile([128, T], BF16)
        vN = act.tile([128, NS * 128], BF16)  # natural [S,D] as 16 s-tiles
        yTj = [act.tile([128, 512], BF16, name=f"yTj{h}") for h in range(HPC)]

        part = [dram.tile([C, 512], BF16, name=f"part{j}") for j in range(NT)]
        rsb = [dram.tile([2 * 128, 512], BF16, name=f"rsb{j}") for j in range(NT)]

        # upfront loads, ordered by first use (nc.sync queue, in-order).
        # xT for chunks j>0 is prefetched inside the j-loop (one chunk ahead)
        # so the per-chunk vN transposes don't queue behind them.
        nc.sync.dma_start(wv_sb[:], wv[:, :])
        nc.sync.dma_start(wk_sb[:], wk[:, :])
        nc.sync.dma_start(wq_sb[:], wq[:, :])
        if "A" in phases:
            for k in range(NK):
                nc.sync.dma_start(
                    xT[k][:, 0:512], xb[k * 128:(k + 1) * 128, 0:512]
                )
        nc.sync.dma_start(tkc[:], wkc[:, :])
        nc.sync.dma_start(tks[:], wks[:, :])
        nc.sync.dma_start(tqc[:], wqc[:, :])
        nc.sync.dma_start(tqs[:], wqs[:, :])
        nc.sync.dma_start(wo_sb[:], wo[:, :])
        nc.sync.dma_start(mnegb[:], mnegp[:, :])
        nc.sync.dma_start(idz[:, 0:128], identp[:, :])

        def norm_rope(ps, wcos, wsin, dest, js):
            """dest[:, js] = rope(rmsnorm-scaled ps); weight folded in tables."""
            raw = work.tile([128, 512], BF16, tag="raw")
            nc.scalar.activation(
                raw[:], ps[:], mybir.ActivationFunctionType.Copy, scale=1.0
            )
            sqr = work.tile([128, 512], BF16, tag="sqr")
            nc.vector.tensor_mul(sqr[:], raw[:], raw[:])
            ssq = P.tile([128, 512], F32, tag="p6")
            nc.tensor.matmul(ssq[0:1, :], ones_colb[:], sqr[:])
            rms = work.tile([1, 512], F32, tag="rms")
            nc.scalar.activation(
                rms[:], ssq[0:1, :], mybir.ActivationFunctionType.Sqrt,
                scale=1.0 / 128.0, bias=eps_col[0:1, :],
            )
            rinv = work.tile([1, 512], BF16, tag="rinv")
            with nc.allow_low_precision(reason="feeds PE broadcast"):
                nc.vector.reciprocal(rinv[:], rms[:])
            rb = P.tile([128, 512], F32, tag="p7")
            nc.tensor.matmul(rb[:], ones_rowb[:], rinv[:])
            qs = work.tile([128, 512], BF16, tag="qs")
            nc.vector.stream_shuffle(qs[:], raw[:], mask=SWAP_MASK)
            t1 = work.tile([128, 512], BF16, tag="t1")
            nc.vector.tensor_mul(t1[:], raw[:], wcos[:, js])
            t2 = work.tile([128, 512], BF16, tag="t2")
            nc.vector.tensor_mul(t2[:], qs[:], wsin[:, js])
            pre = work.tile([128, 512], BF16, tag="pre")
            nc.vector.tensor_add(pre[:], t1[:], t2[:])
            nc.vector.tensor_mul(dest[:, js], pre[:], rb[:])

        def body():
            for j in range(NT):
                js = slice(j * 512, (j + 1) * 512)

                # ===== QKV projection + RMSNorm + RoPE for chunk j =====
                # K and V first so kT/vN are ready when attention starts.
                if "B" in phases:
                    ps_v = P.tile([128, 512], F32, tag="p7")
                    for k in range(NK):
                        nc.tensor.matmul(
                            ps_v[:], wv_sb[:, k * 128:(k + 1) * 128], xT[k][:, js],
                            start=(k == 0), stop=(k == NK - 1),
                        )
                    # v: [D, T]-chunk -> natural [S, D] tiles via XBAR transpose
                    vt = work.tile([128, 512], BF16, tag="vt")
                    nc.scalar.activation(
                        vt[:], ps_v[:], mybir.ActivationFunctionType.Copy, scale=1.0
                    )
                    for u in range(4):
                        s_tile = j * 4 + u
                        nc.sync.dma_start(
                            vN[:, s_tile * 128:(s_tile + 1) * 128],
                            vt[:, u * 128:(u + 1) * 128], transpose=True,
                        )
                    if "A" in phases and j == 0:
                        for k in range(NK):
                            nc.sync.dma_start(
                                xT[k][:, 512:T], xb[k * 128:(k + 1) * 128, 512:T]
                            )
                    ps_k = P.tile([128, 512], F32, tag="p4")
                    for k in range(NK):
                        nc.tensor.matmul(
                            ps_k[:], wk_sb[:, k * 128:(k + 1) * 128], xT[k][:, js],
                            start=(k == 0), stop=(k == NK - 1),
                        )
                    norm_rope(ps_k, tkc, tks, kT, js)
                    ps_q = [
                        P.tile([128, 512], F32, tag=f"p{h}", name=f"psq{h}")
                        for h in range(HPC)
                    ]
                    for h in range(HPC):
                        for k in range(NK):
                            nc.tensor.matmul(
                                ps_q[h][:],
                                wq_sb[:, (h * NK + k) * 128:(h * NK + k + 1) * 128],
                                xT[k][:, js], start=(k == 0), stop=(k == NK - 1),
                            )
                        norm_rope(ps_q[h], tqc, tqs, qT[h], js)

                # ===== attention for chunk j, all 4 local heads =====
                if "D" not in phases:
                    continue
                nblk = 4 * j + 4
                LOOK = 3
                SROT = (0, 1, 5, 6)  # score banks; y double-buffers across heads

                def emit_score(h, i):
                    u = i - 4 * j
                    fs = 0 if u < 0 else 128 * u
                    ps_s = P.tile(
                        [128, 512], F32, tag=f"p{SROT[i % 4]}",
                        name=f"pss{SROT[i % 4]}",
                    )
                    qslice = qT[h][:, j * 512 + fs:(j + 1) * 512]
                    if u >= 0:
                        # diagonal: add -BIG upper-triangle via PE so exp -> 0
                        nc.tensor.matmul(
                            ps_s[:, fs:512], kT[:, i * 128:(i + 1) * 128],
                            qslice, start=True, stop=False,
                        )
                        nc.tensor.matmul(
                            ps_s[:, fs:fs + 128], mnegb[:], idz[:, 0:128],
                            start=False, stop=True,
                        )
                    else:
                        nc.tensor.matmul(
                            ps_s[:, fs:512], kT[:, i * 128:(i + 1) * 128], qslice
                        )
                    et = etp.tile([128, 512], BF16, tag="et")
                    nc.scalar.activation(
                        et[:, fs:512], ps_s[:, fs:512],
                        mybir.ActivationFunctionType.Exp, scale=float(SCALE),
                    )
                    return et, fs

                def make_tail(ps_y, ps_den, h):
                    def tail():
                        rd = work.tile([1, 512], BF16, tag="rd")
                        with nc.allow_low_precision(reason="feeds PE broadcast"):
                            nc.vector.reciprocal(rd[:], ps_den[0:1, :])
                        ps_rb = P.tile([128, 512], F32, tag="p4")
                        nc.tensor.matmul(ps_rb[:], ones_rowb[:], rd[:])
                        yb = work.tile([128, 512], BF16, tag="yb")
                        nc.scalar.activation(
                            yb[:], ps_y[:],
                            mybir.ActivationFunctionType.Copy, scale=1.0,
                        )
                        nc.vector.tensor_mul(yTj[h][:], yb[:], ps_rb[:])
                    return tail

                for h in range(HPC):
                    ps_y = P.tile([128, 512], F32, tag="p2" if h % 2 == 0 else "p7")
                    ps_den = P.tile([128, 512], F32, tag="p3")
                    pend = {}
                    for i in range(min(LOOK, nblk)):
                        pend[i] = emit_score(h, i)
                    for i in range(nblk):
                        if i + LOOK < nblk:
                            pend[i + LOOK] = emit_score(h, i + LOOK)
                        et, fs = pend.pop(i)
                        st = dict(start=(i == 0), stop=(i == nblk - 1))
                        nc.tensor.matmul(
                            ps_y[:, fs:512], vN[:, i * 128:(i + 1) * 128],
                            et[:, fs:512], **st,
                        )
                        nc.tensor.matmul(
                            ps_den[0:1, fs:512], ones_colb[:], et[:, fs:512], **st
                        )
                    make_tail(ps_y, ps_den, h)()

                # ===== o_proj partial for chunk j + ReduceScatter =====
                if "F" not in phases:
                    continue
                for mp in range(NM // 2):
                    ob2 = work.tile([128, 1024], BF16, tag=f"ob{mp % 3}")
                    for half in range(2):
                        m = 2 * mp + half
                        ps_o = P.tile([128, 512], F32, tag=f"p{5 + m % 2}")
                        for h in range(HPC):
                            nc.tensor.matmul(
                                ps_o[:],
                                wo_sb[:, (m * HPC + h) * 128:(m * HPC + h + 1) * 128],
                                yTj[h][:], start=(h == 0), stop=(h == HPC - 1),
                            )
                        dst = ob2[:, half * 512:(half + 1) * 512]
                        nc.vector.tensor_copy(dst, ps_o[:])
                    nc.sync.dma_start(
                        part[j][2 * mp * 128:(2 * mp + 2) * 128, :].rearrange(
                            "(two r) c -> r two c", two=2
                        ),
                        ob2[:],
                    )
                if collectives:
                    nc.gpsimd.collective_compute(
                        "ReduceScatter", mybir.AluOpType.add, replica_groups=rg,
                        ins=[part[j][:].opt()], outs=[rsb[j][:].opt()],
                    )

        if bench_reps:
            with tc.For_i(0, bench_reps, 1):
                body()
        else:
            body()
            if collectives and "F" in phases:
                for j in range(NT):
                    nc.sync.dma_start(rs_out[j][:, :], rsb[j][:])

    split_multiwaits(nc)
    return nc


# ---------------------------------------------------------------------------
# host side
# ---------------------------------------------------------------------------

_RUNNER_CACHE = None


def _make_runner(nc, n_cores=NCORES):
    """Build the sharded jit once; returns run(in_maps) -> list of out dicts."""
    import jax
    from jax.sharding import Mesh, NamedSharding, PartitionSpec
    from jax.experimental.shard_map import shard_map
    from concourse import bass2jax
    from concourse.bass2jax import _bass_exec_p, partition_id_tensor

    bass2jax.install_neuronx_cc_hook()

    partition_name = nc.partition_id_tensor.name if nc.partition_id_tensor else None
    in_names, out_names, out_avals, zero_outs = [], [], [], []
    for alloc in nc.m.functions[0].allocations:
        if not isinstance(alloc, mybir.MemoryLocationSet):
            continue
        name = alloc.memorylocations[0].name
        if alloc.kind == "ExternalInput":
            if name != partition_name:
                in_names.append(name)
        elif alloc.kind == "ExternalOutput":
            out_names.append(name)
            shape = tuple(alloc.tensor_shape)
            dtype = mybir.dt.np(alloc.dtype)
            out_avals.append(jax.core.ShapedArray(shape, dtype))
            zero_outs.append(np.zeros(shape, dtype))
    n_params = len(in_names)
    n_outs = len(out_avals)
    all_in_names = list(in_names) + list(out_names)
    if partition_name is not None:
        all_in_names.append(partition_name)
    donate = tuple(range(n_params, n_params + n_outs))

    def _body(*args):
        operands = list(args)
        if partition_name is not None:
            operands.append(partition_id_tensor())
        outs = _bass_exec_p.bind(
            *operands,
            out_avals=tuple(out_avals),
            in_names=tuple(all_in_names),
            out_names=tuple(out_names),
            lowering_input_output_aliases=(),
            sim_require_finite=True,
            sim_require_nnan=True,
            nc=nc,
        )
        return tuple(outs)

    devices = jax.devices()[:n_cores]
    mesh = Mesh(np.asarray(devices), ("core",))
    sharded = jax.jit(
        shard_map(
            _body, mesh=mesh,
            in_specs=(PartitionSpec("core"),) * (n_params + n_outs),
            out_specs=(PartitionSpec("core"),) * n_outs,
            check_rep=False,
        ),
        donate_argnums=donate,
        keep_unused=True,
    )
    shard = NamedSharding(mesh, PartitionSpec("core"))
    zshapes = [((n_cores * z.shape[0],) + z.shape[1:], z.dtype) for z in zero_outs]

    def run(in_maps):
        concat_in = [
            jax.device_put(
                np.concatenate(
                    [np.asarray(in_maps[c][n]) for c in range(n_cores)], axis=0
                ),
                shard,
            )
            for n in in_names
        ]
        zs = [jax.device_put(np.zeros(s, d), shard) for s, d in zshapes]
        outs = sharded(*concat_in, *zs)
        return [
            {
                name: np.asarray(outs[i]).reshape(n_cores, *out_avals[i].shape)[c]
                for i, name in enumerate(out_names)
            }
            for c in range(n_cores)
        ]

    return run


def _get_runner():
    global _RUNNER_CACHE
    if _RUNNER_CACHE is None:
        _RUNNER_CACHE = _make_runner(build_program())
    return _RUNNER_CACHE


def make_inputs(x, input_pos, Wq, Wk, Wv, Wo, q_norm_w, k_norm_w):
    """Host-side sharding / layout prep. Returns per-core input maps."""
    bf16 = _bf16()
    x2d = np.ascontiguousarray(
        np.asarray(x, np.float32).reshape(T, C).T
    ).astype(bf16)  # pre-transposed: [C, T]
    Wq = np.asarray(Wq, np.float32)
    Wk = np.asarray(Wk, np.float32)
    Wv = np.asarray(Wv, np.float32)
    Wo = np.asarray(Wo, np.float32)
    q_norm_w = np.asarray(q_norm_w, np.float32)
    k_norm_w = np.asarray(k_norm_w, np.float32)
    pos = np.asarray(input_pos, np.float32)

    # interleaved head-dim permutation: [0, 64, 1, 65, ...]
    perm = np.empty(128, np.int64)
    perm[0::2] = np.arange(64)
    perm[1::2] = np.arange(64) + 64
    pswap = np.arange(128) ^ 1  # adjacent-pair swap of interleaved rows

    # rope tables in interleaved layout (sign of the rotate-half folded in),
    # with the rmsnorm weight folded in: the even/odd rows of the sin table
    # carry the weight of the PAIRED row (the shuffled operand).
    inv_freq = (THETA ** (-(np.arange(0, D, 2, dtype=np.float32)) / D)).astype(
        np.float32
    )
    fr = pos[:, None] * inv_freq[None, :]  # [T, 64]
    cos = np.cos(fr).astype(np.float32).T  # [64, T]
    sin = np.sin(fr).astype(np.float32).T
    cos_il = np.empty((128, T), np.float32)
    cos_il[0::2] = cos
    cos_il[1::2] = cos
    sin_eff = np.empty((128, T), np.float32)
    sin_eff[0::2] = -sin
    sin_eff[1::2] = sin

    def fold(w):
        wp = w[perm]
        wc = np.ascontiguousarray(cos_il * wp[:, None]).astype(bf16)
        ws = np.ascontiguousarray(sin_eff * wp[pswap][:, None]).astype(bf16)
        return wc, ws

    wqc_h, wqs_h = fold(q_norm_w)
    wkc_h, wks_h = fold(k_norm_w)

    ident_h = np.eye(128, dtype=np.float32).astype(bf16)
    # mneg[r, c] = -BIG iff r < c; with rhs=[I|0] this adds -BIG to score[p, x]
    # for x < p (future positions) so exp underflows to exactly 0.
    rr, cc2 = np.meshgrid(np.arange(128), np.arange(128), indexing="ij")
    mneg_h = np.where(rr < cc2, np.float32(-1e30), np.float32(0)).astype(bf16)

    Wq4 = Wq.reshape(N_HEAD, D, C)
    Wk4 = Wk.reshape(N_KV, D, C)
    Wv4 = Wv.reshape(N_KV, D, C)

    in_maps = []
    for c in range(NCORES):
        g = c // 2
        Wc = Wq4[HPC * c:HPC * (c + 1)][:, perm, :]  # [4, 128, C]
        wq_host = np.ascontiguousarray(
            Wc.reshape(HPC, 128, NK, 128).transpose(3, 0, 2, 1).reshape(128, -1)
        ).astype(bf16)
        wk_host = np.ascontiguousarray(
            Wk4[g][perm].reshape(128, NK, 128).transpose(2, 1, 0).reshape(128, -1)
        ).astype(bf16)
        wv_host = np.ascontiguousarray(
            Wv4[g].reshape(128, NK, 128).transpose(2, 1, 0).reshape(128, -1)
        ).astype(bf16)
        # o_proj lhsT per (m-tile, local head): wo_host[d, (m*4+h)*128 + i]
        # = Wo[128m+i, 512c + 128h + d]
        WoC = Wo[:, 512 * c:512 * (c + 1)]  # [2048, 512]
        wo_host = np.ascontiguousarray(
            WoC.reshape(NM, 128, HPC, 128).transpose(3, 0, 2, 1).reshape(128, -1)
        ).astype(bf16)
        in_maps.append(
            {
                "xb": x2d,
                "wq": wq_host,
                "wk": wk_host,
                "wv": wv_host,
                "wo": wo_host,
                "wqc": wqc_h,
                "wqs": wqs_h,
                "wkc": wkc_h,
                "wks": wks_h,
                "identp": ident_h,
                "mnegp": mneg_h,
            }
        )
    return in_maps


def kernel(x, input_pos, Wq, Wk, Wv, Wo, q_norm_w, k_norm_w):
    run = _get_runner()
    in_maps = make_inputs(x, input_pos, Wq, Wk, Wv, Wo, q_norm_w, k_norm_w)
    results = run(in_maps)
    out = np.empty((1, T, C), np.float32)
    for c in range(NCORES):
        for j in range(NT):
            out[0][j * 512:(j + 1) * 512, 256 * c:256 * (c + 1)] = (
                results[c][f"rs{j}"].astype(np.float32).T
            )
    return out
